# revision 1
# baseline (speedup 1.0000x reference)
"""Trainium2 Bass kernel for nn_CausalMoE.

Reference computation (B=2, S=2048, H=2048, G=16, GH=8, FFN=8192):
  cv        = tanh(hs @ P_extract)                        [N,G]   N = B*S = 4096
  pi        = cv @ A                                      [N,G]
  h[:,m,:]  = cv @ W1[m,:G,:] + pi[:,m,None]*W1[m,G,:] + b1[m]
  h         = gelu(h)  (exact erf gelu)                   [N,G,GH]
  effects   = sum_k h[:,m,k] W2[m,k] + b2[m]              [N,G]
  modified  = hs + 0.5 * effects @ P_route                [N,H]
  ffn_h     = gelu(modified @ ffn_w1 + ffn_b1)            [N,F]
  out       = ffn_h @ ffn_w2 + ffn_b2                     [N,H]

Strategy: pure data-parallel over the 8 NeuronCores (512 tokens/core),
weights replicated.  Everything is computed feature-major (activations
stored transposed, [feature, token]) so every matmul has its contraction
dim on partitions with weights as the stationary operand.  The host
shards hs in transposed layout and the gather transposes the output
shards back, so the kernel needs no on-chip transposes at all -- the PE
does nothing but the actual matmuls.  All matmuls run in float32r
(rounded-fp32 PE mode, ~2e-4 matmul relerr, bf16-class speed at 512-wide
moving operands; plain fp32 is 4x slower).  The tiny causal-mechanism
loop is folded into three small matmuls via host-side weight
restructuring.  FFN runs in 4 F-blocks of 2048 with an f32r SBUF output
accumulator (6 PSUM banks pipeline the accumulation/eviction).  The big
weights are re-tiled on the host so every weight DMA is a single
fully-contiguous 1 MiB read (8 KiB per partition), which lets DMA keep
up with the ~290 GB/s streaming demand of the PE.  Measured: ~513 us HW
exec, PE 94.4% busy, the FFN matmul stream at the 227 ns/matmul f32r
issue-rate limit with ~2 us total excess; max rel err 3.3e-3 vs the
fp32 reference.
"""
import sys

sys.path.insert(0, "/opt/trn_rl_repo")

import numpy as np

import concourse.bacc as bacc
import concourse.mybir as mybir
import concourse.tile as tile
from concourse.bass_utils import run_bass_kernel_spmd

F32 = mybir.dt.float32
F32R = mybir.dt.float32r
AF = mybir.ActivationFunctionType

B, S, H = 2, 2048, 2048
G, GH, F = 16, 8, 8192
N_CORES = 8
NTOK = B * S              # 4096 tokens total
T = NTOK // N_CORES       # 512 tokens per core
KO = H // 128             # 16 contraction tiles over H
FO = F // 128             # 64 F tiles
TO = T // 128             # 4 token tiles
NBLK = 4                  # F blocks
FPB = FO // NBLK          # 16 F tiles per block

_CACHE = {}


def _build():
    nc = bacc.Bacc("TRN2", target_bir_lowering=False, debug=False)
    # host-side shard layout: xtd = hs_shard.T  [H, T] (feature-major)
    xtd = nc.dram_tensor("xtd", [H, T], F32, kind="ExternalInput").ap()
    pe = nc.dram_tensor("pe", [128, KO, G], F32, kind="ExternalInput").ap()
    adj = nc.dram_tensor("adj", [G, G], F32, kind="ExternalInput").ap()
    w1a = nc.dram_tensor("w1a", [G, G * GH], F32, kind="ExternalInput").ap()
    w1b = nc.dram_tensor("w1b", [G, G * GH], F32, kind="ExternalInput").ap()
    b1f = nc.dram_tensor("b1f", [G * GH, 1], F32, kind="ExternalInput").ap()
    w2bd = nc.dram_tensor("w2bd", [G * GH, G], F32, kind="ExternalInput").ap()
    b2s = nc.dram_tensor("b2s", [G, 1], F32, kind="ExternalInput").ap()
    pr = nc.dram_tensor("pr", [G, H], F32, kind="ExternalInput").ap()
    # host-retiled: fw1t[fo, p, ko, f] = ffn_w1[ko*128+p, fo*128+f]
    fw1 = nc.dram_tensor("fw1", [FO, 128, KO, 128], F32, kind="ExternalInput").ap()
    fb1 = nc.dram_tensor("fb1", [128, FO], F32, kind="ExternalInput").ap()
    # host-retiled: fw2t[ho, b, p, j, h] = ffn_w2[(b*FPB+j)*128+p, ho*128+h]
    fw2 = nc.dram_tensor(
        "fw2", [KO, NBLK, 128, FPB, 128], F32, kind="ExternalInput"
    ).ap()
    fb2 = nc.dram_tensor("fb2", [128, KO], F32, kind="ExternalInput").ap()
    # output stays feature-major [H, T]; the host gather transposes
    out = nc.dram_tensor("out", [H, T], F32, kind="ExternalOutput").ap()

    with tile.TileContext(nc) as tc:
        with (
            tc.tile_pool(name="const", bufs=1) as const,
            tc.tile_pool(name="xt", bufs=1) as xtp,
            tc.tile_pool(name="h1", bufs=1) as h1p,
            tc.tile_pool(name="oacc", bufs=1) as oap,
            tc.tile_pool(name="w1", bufs=6) as w1p,
            tc.tile_pool(name="w2", bufs=5) as w2p,
            tc.tile_pool(name="sm", bufs=1) as smp,
            tc.tile_pool(name="mm", bufs=6, space="PSUM") as mmp,
        ):
            # explicit zero tile for activation biases: a float bias would
            # synthesize a const-AP pool whose TENSOR_LOAD sits in the
            # serialized kernel preamble (~2.7us)
            zz = const.tile([G, 1], F32)
            nc.gpsimd.memset(zz[:], 0.0)

            # PE clock warm-up: HAM keeps the PE throttled at 1.2 GHz until
            # ~3.4us of sustained matmul activity.  The PE is otherwise idle
            # while the xT shard DMAs in, so without this the extraction,
            # routing and first ~15 FFN matmuls all run at half speed
            # (measured: K=8/8 only fired at t~44us).  A few junk fp32
            # matmuls on a zeroed scratch tile lift the throttle by t~13us.
            scr = const.tile([128, T], F32)
            nc.gpsimd.memset(scr[:], 0.0)
            jp = mmp.tile([128, T], F32, tag="mm")

            def pe_keepalive(n, width=256):
                # junk matmuls that keep the PE's HAM activity window busy
                # across known dependency stalls (idle >3.4us re-throttles)
                for _ in range(n):
                    nc.tensor.matmul(
                        jp[:, 0:width], scr[:, 0:128], scr[:, 0:width],
                        start=True, stop=True,
                    )

            pe_keepalive(4, width=T)   # ~4us: trip the un-throttle window

            # warm the ACT Tanh+Gelu LUTs during the xT load, so the
            # ~1.3us table loads are off the small-chain critical path
            act_warm = const.tile([1, 2], F32)
            nc.scalar.activation(act_warm[:, 0:1], zz[0:1, :], AF.Tanh,
                                 bias=zz[0:1, :])
            nc.scalar.activation(act_warm[:, 1:2], zz[0:1, :], AF.Gelu,
                                 bias=zz[0:1, :])

            # small consts on the gpsimd DMA queue so the sync queue is
            # free for x chunks + weight streaming from t=0
            pe_sb = const.tile([128, KO, G], F32R)
            nc.gpsimd.dma_start(pe_sb[:], pe.bitcast(F32R))
            adj_sb = const.tile([G, G], F32R)
            nc.gpsimd.dma_start(adj_sb[:], adj.bitcast(F32R))
            w1a_sb = const.tile([G, G * GH], F32R)
            nc.gpsimd.dma_start(w1a_sb[:], w1a.bitcast(F32R))
            w1b_sb = const.tile([G, G * GH], F32R)
            nc.gpsimd.dma_start(w1b_sb[:], w1b.bitcast(F32R))
            b1f_sb = const.tile([G * GH, 1], F32)
            nc.gpsimd.dma_start(b1f_sb[:], b1f)
            w2bd_sb = const.tile([G * GH, G], F32R)
            nc.gpsimd.dma_start(w2bd_sb[:], w2bd.bitcast(F32R))
            b2s_sb = const.tile([G, 1], F32)
            nc.gpsimd.dma_start(b2s_sb[:], b2s)
            pr_sb = const.tile([G, H], F32R)
            nc.gpsimd.dma_start(pr_sb[:], pr.bitcast(F32R))
            fb1_sb = const.tile([128, FO], F32)
            nc.gpsimd.dma_start(fb1_sb[:], fb1)
            fb2_sb = const.tile([128, KO], F32)
            nc.gpsimd.dma_start(fb2_sb[:], fb2)

            # ---- load feature-major xT [128, KO, T] straight from HBM ----
            # (the host shards hs in transposed layout, so no PE transposes
            # or PSUM evictions are needed on the input side at all)
            xT = xtp.tile([128, KO, T], F32R)
            xtd_t = xtd.bitcast(F32R).rearrange("(ko p) t -> p ko t", p=128)
            for g in range(4):
                nc.sync.dma_start(
                    xT[:, g * 4:(g + 1) * 4, :], xtd_t[:, g * 4:(g + 1) * 4, :]
                )

            # ---- causal-variable extraction: cv^T = tanh(Pe^T @ x^T) ----
            cv_ps = mmp.tile([128, T], F32, tag="mm")
            for ko in range(KO):
                nc.tensor.matmul(
                    cv_ps[0:G, :], pe_sb[:, ko, :], xT[:, ko, :],
                    start=(ko == 0), stop=(ko == KO - 1),
                )
                if ko in (3, 7, 11):
                    # absorb the wait for the next xT DMA group so the
                    # HAM activity window stays busy (idle re-throttles
                    # the PE to 1.2 GHz for routing + early FFN1)
                    pe_keepalive(4, width=128)
            cvt_sb = smp.tile([G, T], F32R, tag="cv")
            nc.scalar.activation(cvt_sb[:], cv_ps[0:G, :], AF.Tanh,
                                 bias=zz[:])
            pe_keepalive(3)

            # pi^T = A^T @ cv^T
            pi_ps = mmp.tile([128, T], F32, tag="mm")
            nc.tensor.matmul(
                pi_ps[0:G, :], adj_sb[:], cvt_sb[:], start=True, stop=True
            )
            pit_sb = smp.tile([G, T], F32R, tag="pi")
            nc.vector.tensor_copy(pit_sb[:], pi_ps[0:G, :])

            # mechanism hidden: gelu(W1a^T @ cv + W1b^T @ pi + b1)
            h_ps = mmp.tile([128, T], F32, tag="mm")
            nc.tensor.matmul(h_ps[:], w1a_sb[:], cvt_sb[:], start=True, stop=False)
            nc.tensor.matmul(h_ps[:], w1b_sb[:], pit_sb[:], start=False, stop=True)
            hm_sb = smp.tile([G * GH, T], F32R, tag="hm")
            nc.scalar.activation(hm_sb[:], h_ps[:], AF.Gelu, bias=b1f_sb[:])
            pe_keepalive(3)

            # effects*0.5 = W2bd^T @ hm + b2*0.5
            eff_ps = mmp.tile([128, T], F32, tag="mm")
            nc.tensor.matmul(
                eff_ps[0:G, :], w2bd_sb[:], hm_sb[:], start=True, stop=True
            )
            # bias-add on DVE: keeps the ACT LUT on Gelu (no table reload)
            effs_sb = smp.tile([G, T], F32R, tag="eff")
            nc.vector.tensor_scalar_add(effs_sb[:], eff_ps[0:G, :], b2s_sb[:])
            pe_keepalive(4)

            # ---- modified^T = x^T + P_route^T @ effs  (in place on xT) ----
            for ho in range(KO):
                md = mmp.tile([128, T], F32, tag="mm")
                nc.tensor.matmul(
                    md[:], pr_sb[:, ho * 128:(ho + 1) * 128], effs_sb[:],
                    start=True, stop=True,
                )
                nc.vector.tensor_add(xT[:, ho, :], xT[:, ho, :], md[:])

            # ---- FFN in 4 F-blocks, f32r SBUF accumulator for layer 2 ----
            out_acc = oap.tile([128, KO, T], F32R)

            out_t = out.bitcast(F32R).rearrange("(ho p) t -> p ho t", p=128)

            for b in range(NBLK):
                h1b = h1p.tile([128, FPB, T], F32R, tag="h1")
                for j in range(FPB):
                    fo = b * FPB + j
                    wt = w1p.tile([128, KO, 128], F32R, tag="w1")
                    nc.sync.dma_start(wt[:], fw1[fo].bitcast(F32R))
                    pf = mmp.tile([128, T], F32, tag="mm")
                    for ko in range(KO):
                        nc.tensor.matmul(
                            pf[:], wt[:, ko, :], xT[:, ko, :],
                            start=(ko == 0), stop=(ko == KO - 1),
                        )
                    nc.scalar.activation(
                        h1b[:, j, :], pf[:], AF.Gelu, bias=fb1_sb[:, fo:fo + 1]
                    )
                for ho in range(KO):
                    w2t = w2p.tile([128, FPB, 128], F32R, tag="w2")
                    nc.sync.dma_start(w2t[:], fw2[ho, b].bitcast(F32R))
                    po = mmp.tile([128, T], F32, tag="mm")
                    for j in range(FPB):
                        nc.tensor.matmul(
                            po[:], w2t[:, j, :], h1b[:, j, :],
                            start=(j == 0), stop=(j == FPB - 1),
                        )
                    if b == 0:
                        nc.vector.tensor_scalar_add(
                            out_acc[:, ho, :], po[:], fb2_sb[:, ho:ho + 1]
                        )
                    else:
                        nc.vector.tensor_add(
                            out_acc[:, ho, :], out_acc[:, ho, :], po[:]
                        )
                    if b == NBLK - 1:
                        # store this H-tile feature-major; host transposes
                        nc.sync.dma_start(out_t[:, ho, :], out_acc[:, ho, :])

    nc.compile()
    return nc


def _prep(inputs):
    """Host-side restructuring of weights + sharding."""
    hs = np.ascontiguousarray(np.asarray(inputs["hidden_states"], np.float32))
    W1 = np.asarray(inputs["W1"], np.float32)
    b1 = np.asarray(inputs["b1"], np.float32)
    W2 = np.asarray(inputs["W2"], np.float32)
    b2 = np.asarray(inputs["b2"], np.float32)

    w1a = np.ascontiguousarray(
        W1[:, :G, :].transpose(1, 0, 2).reshape(G, G * GH)
    )
    w1b = np.zeros((G, G * GH), np.float32)
    for m in range(G):
        w1b[m, m * GH:(m + 1) * GH] = W1[m, G, :]
    b1f = b1.reshape(G * GH, 1)
    w2bd = np.zeros((G * GH, G), np.float32)
    for m in range(G):
        w2bd[m * GH:(m + 1) * GH, m] = 0.5 * W2[m, :]
    b2s = (0.5 * b2).reshape(G, 1)

    pe = np.asarray(inputs["P_extract"], np.float32)
    # pe[h, g] -> [p, ko, g] with h = ko*128 + p
    pe_t = np.ascontiguousarray(pe.reshape(KO, 128, G).transpose(1, 0, 2))

    fw1 = np.asarray(inputs["ffn_w1"], np.float32)
    # fw1[ko*128+p, fo*128+f] -> [fo, p, ko, f]
    fw1_t = np.ascontiguousarray(
        fw1.reshape(KO, 128, FO, 128).transpose(2, 1, 0, 3)
    )
    fw2 = np.asarray(inputs["ffn_w2"], np.float32)
    # fw2[(b*FPB+j)*128+p, ho*128+h] -> [ho, b, p, j, h]
    fw2_t = np.ascontiguousarray(
        fw2.reshape(NBLK, FPB, 128, KO, 128).transpose(3, 0, 2, 1, 4)
    )

    common = {
        "pe": pe_t,
        "adj": np.ascontiguousarray(np.asarray(inputs["causal_adjacency"], np.float32)),
        "w1a": w1a,
        "w1b": w1b,
        "b1f": np.ascontiguousarray(b1f),
        "w2bd": w2bd,
        "b2s": np.ascontiguousarray(b2s),
        "pr": np.ascontiguousarray(np.asarray(inputs["P_route"], np.float32)),
        "fw1": fw1_t,
        "fb1": np.ascontiguousarray(
            np.asarray(inputs["ffn_b1"], np.float32).reshape(FO, 128).T
        ),
        "fw2": fw2_t,
        "fb2": np.ascontiguousarray(
            np.asarray(inputs["ffn_b2"], np.float32).reshape(KO, 128).T
        ),
    }
    toks = hs.reshape(NTOK, H)
    in_maps = []
    for c in range(N_CORES):
        m = dict(common)
        m["xtd"] = np.ascontiguousarray(toks[c * T:(c + 1) * T].T)
        in_maps.append(m)
    return in_maps


def run(inputs, trace=False):
    """Returns (full output [B,S,H] fp32, BassKernelResults)."""
    if "nc" not in _CACHE:
        _CACHE["nc"] = _build()
    nc = _CACHE["nc"]
    in_maps = _prep(inputs)
    res = run_bass_kernel_spmd(
        nc, in_maps, core_ids=list(range(N_CORES)), trace=trace
    )
    full = np.empty((NTOK, H), np.float32)
    for c in range(N_CORES):
        full[c * T:(c + 1) * T] = res.results[c]["out"].T
    return full.reshape(B, S, H), res


def kernel(**inputs):
    full, _ = run(inputs, trace=False)
    return full



# revision 16
# speedup vs baseline: 1.0541x; 1.0541x over previous
"""Trainium2 Bass kernel for nn_CausalMoE.

Reference computation (B=2, S=2048, H=2048, G=16, GH=8, FFN=8192):
  cv        = tanh(hs @ P_extract)                        [N,G]   N = B*S = 4096
  pi        = cv @ A                                      [N,G]
  h[:,m,:]  = cv @ W1[m,:G,:] + pi[:,m,None]*W1[m,G,:] + b1[m]
  h         = gelu(h)  (exact erf gelu)                   [N,G,GH]
  effects   = sum_k h[:,m,k] W2[m,k] + b2[m]              [N,G]
  modified  = hs + 0.5 * effects @ P_route                [N,H]
  ffn_h     = gelu(modified @ ffn_w1 + ffn_b1)            [N,F]
  out       = ffn_h @ ffn_w2 + ffn_b2                     [N,H]

Strategy: pure data-parallel over the 8 NeuronCores (512 tokens/core),
weights replicated.  Everything is computed feature-major (activations
stored transposed, [feature, token]) so every matmul has its contraction
dim on partitions with weights as the stationary operand; the host
shards hs transposed and the gather transposes the output back, so the
kernel needs no on-chip transposes.  The FFN runs entirely in fp16
(e5m10): same 1-cycle/row PE speed class as bf16, FWL weight loads
(97 ns vs 187 ns for fp32 -- the f32r stream was weight-load bound at
227 ns/MM; fp16 hits the 216 ns N=512 issue floor), half the weight DMA
of f32r, and ~11-bit mantissas keep the max rel err ~6e-3.  The tiny
causal-mechanism chain stays f32r (its weights are folded into three
small matmuls via host-side restructuring).

The serial prologue is eliminated with a low-rank split of FFN layer 1:
modified @ ffn_w1 = hs @ ffn_w1 + effects @ (0.5 P_route @ ffn_w1),
where prw1 = P_route @ ffn_w1 [16, 8192] is precomputed on the host.
The first E=4 F-tiles of FFN1 run on raw hs (their rank-16 correction is
one K=16 matmul into the same PSUM accumulation before the gelu), so the
big matmul stream starts the moment the first x chunk + weight tile land
(~1 us) and the extraction/mechanism chain, routing matmuls and
modified-x evictions all hide inside it.  modified is written to a
separate tile (xT stays read-only) so the DVE evictions have no
ordering hazard against the raw-x matmuls.  FFN1 of block b+1 is
interleaved with FFN2 of block b to cover the last-gelu ACT latency at
each block boundary.  No PE idle gaps => the HAM clock gate stays at
2.4 GHz after the initial ~3.4 us ramp, with no junk keepalive matmuls.
"""
import sys

sys.path.insert(0, "/opt/trn_rl_repo")

import numpy as np

import concourse.bacc as bacc
import concourse.mybir as mybir
import concourse.tile as tile
from concourse.bass_utils import run_bass_kernel_spmd

F32 = mybir.dt.float32
F32R = mybir.dt.float32r
F16 = mybir.dt.float16
AF = mybir.ActivationFunctionType

B, S, H = 2, 2048, 2048
G, GH, F = 16, 8, 8192
N_CORES = 8
NTOK = B * S              # 4096 tokens total
T = NTOK // N_CORES       # 512 tokens per core
KO = H // 128             # 16 contraction tiles over H
FO = F // 128             # 64 F tiles
NBLK = 4                  # F blocks
FPB = FO // NBLK          # 16 F tiles per block
NRAW = 4                  # leading F-tiles computed on raw hs + rank-16 fixup

_CACHE = {}


def _build():
    nc = bacc.Bacc("TRN2", target_bir_lowering=False, debug=False)
    # host-side shard layout: xtd = hs_shard.T  [H, T] (feature-major, fp16)
    xtd = nc.dram_tensor("xtd", [H, T], F16, kind="ExternalInput").ap()
    pe = nc.dram_tensor("pe", [128, KO, G], F16, kind="ExternalInput").ap()
    adj = nc.dram_tensor("adj", [G, G], F32, kind="ExternalInput").ap()
    w1a = nc.dram_tensor("w1a", [G, G * GH], F32, kind="ExternalInput").ap()
    w1b = nc.dram_tensor("w1b", [G, G * GH], F32, kind="ExternalInput").ap()
    b1f = nc.dram_tensor("b1f", [G * GH, 1], F32, kind="ExternalInput").ap()
    w2bd = nc.dram_tensor("w2bd", [G * GH, G], F32, kind="ExternalInput").ap()
    b2s = nc.dram_tensor("b2s", [G, 1], F32, kind="ExternalInput").ap()
    pr = nc.dram_tensor("pr", [G, H], F16, kind="ExternalInput").ap()
    # prw1 = P_route @ ffn_w1, [g, fo, f]; correction weights for raw tiles
    prw1 = nc.dram_tensor("prw1", [G, NRAW, 128], F16, kind="ExternalInput").ap()
    # host-retiled: fw1t[fo, p, ko, f] = ffn_w1[ko*128+p, fo*128+f]  (fp16)
    fw1 = nc.dram_tensor("fw1", [FO, 128, KO, 128], F16, kind="ExternalInput").ap()
    fb1 = nc.dram_tensor("fb1", [128, FO], F32, kind="ExternalInput").ap()
    # host-retiled: fw2t[ho, b, p, j, h] = ffn_w2[(b*FPB+j)*128+p, ho*128+h]
    fw2 = nc.dram_tensor(
        "fw2", [KO, NBLK, 128, FPB, 128], F16, kind="ExternalInput"
    ).ap()
    fb2 = nc.dram_tensor("fb2", [128, KO], F32, kind="ExternalInput").ap()
    # output stays feature-major [H, T]; the host gather transposes
    out = nc.dram_tensor("out", [H, T], F32, kind="ExternalOutput").ap()

    with tile.TileContext(nc) as tc:
        with (
            tc.tile_pool(name="const", bufs=1) as const,
            tc.tile_pool(name="xt", bufs=1) as xtp,
            tc.tile_pool(name="mod", bufs=1) as modp,
            tc.tile_pool(name="h1", bufs=2) as h1p,
            tc.tile_pool(name="oacc", bufs=1) as oap,
            tc.tile_pool(name="w1", bufs=6) as w1p,
            tc.tile_pool(name="w2", bufs=5) as w2p,
            tc.tile_pool(name="sm", bufs=1) as smp,
            tc.tile_pool(name="mm", bufs=4, space="PSUM") as mmp,
            tc.tile_pool(name="md", bufs=2, space="PSUM") as mdp,
            tc.tile_pool(name="ch", bufs=2, space="PSUM") as chp,
        ):
            # explicit zero tile for activation biases: a float bias would
            # synthesize a const-AP pool whose TENSOR_LOAD sits in the
            # serialized kernel preamble (~2.7us)
            zz = const.tile([G, 1], F32)
            nc.gpsimd.memset(zz[:], 0.0)

            # warm the ACT Tanh+Gelu LUTs while DMAs land, so the ~1.3us
            # table loads are off the small-chain critical path
            act_warm = const.tile([1, 2], F32)
            nc.scalar.activation(act_warm[:, 0:1], zz[0:1, :], AF.Tanh,
                                 bias=zz[0:1, :])
            nc.scalar.activation(act_warm[:, 1:2], zz[0:1, :], AF.Gelu,
                                 bias=zz[0:1, :])

            # small consts on the gpsimd DMA queue; the sync queue carries
            # the x chunks + big weight stream from t=0
            pe_sb = const.tile([128, KO, G], F16)
            nc.gpsimd.dma_start(pe_sb[:], pe)
            adj_sb = const.tile([G, G], F32R)
            nc.gpsimd.dma_start(adj_sb[:], adj.bitcast(F32R))
            w1a_sb = const.tile([G, G * GH], F32R)
            nc.gpsimd.dma_start(w1a_sb[:], w1a.bitcast(F32R))
            w1b_sb = const.tile([G, G * GH], F32R)
            nc.gpsimd.dma_start(w1b_sb[:], w1b.bitcast(F32R))
            b1f_sb = const.tile([G * GH, 1], F32)
            nc.gpsimd.dma_start(b1f_sb[:], b1f)
            w2bd_sb = const.tile([G * GH, G], F32R)
            nc.gpsimd.dma_start(w2bd_sb[:], w2bd.bitcast(F32R))
            b2s_sb = const.tile([G, 1], F32)
            nc.gpsimd.dma_start(b2s_sb[:], b2s)
            fb1_sb = const.tile([128, FO], F32)
            nc.gpsimd.dma_start(fb1_sb[:], fb1)
            fb2_sb = const.tile([128, KO], F32)
            nc.gpsimd.dma_start(fb2_sb[:], fb2)

            # ---- feature-major xT [128, KO, T], split across two queues ----
            # (read-only for the whole kernel: raw tiles + routing read it;
            # modified goes to a separate tile, so no version hazards)
            xT = xtp.tile([128, KO, T], F16)
            xtd_t = xtd.rearrange("(ko p) t -> p ko t", p=128)
            # sync gets ko [0:2],[4:6],[8:10],[12:14]; gpsimd the others
            nc.sync.dma_start(xT[:, 0:2, :], xtd_t[:, 0:2, :])
            nc.sync.dma_start(xT[:, 4:6, :], xtd_t[:, 4:6, :])
            nc.gpsimd.dma_start(xT[:, 2:4, :], xtd_t[:, 2:4, :])
            nc.gpsimd.dma_start(xT[:, 6:8, :], xtd_t[:, 6:8, :])
            nc.gpsimd.dma_start(xT[:, 10:12, :], xtd_t[:, 10:12, :])
            nc.gpsimd.dma_start(xT[:, 14:16, :], xtd_t[:, 14:16, :])

            # first raw-tile weights early on the sync queue
            wts = {}
            wts[0] = w1p.tile([128, KO, 128], F16, tag="w1", name="wt0")
            nc.sync.dma_start(wts[0][:], fw1[0])
            nc.sync.dma_start(xT[:, 8:10, :], xtd_t[:, 8:10, :])
            wts[1] = w1p.tile([128, KO, 128], F16, tag="w1", name="wt1")
            nc.sync.dma_start(wts[1][:], fw1[1])
            nc.sync.dma_start(xT[:, 12:14, :], xtd_t[:, 12:14, :])
            wts[2] = w1p.tile([128, KO, 128], F16, tag="w1", name="wt2")
            nc.sync.dma_start(wts[2][:], fw1[2])

            # routing / correction weights after the x chunks (needed ~10us+)
            pr_sb = const.tile([G, H], F16)
            nc.gpsimd.dma_start(pr_sb[:], pr)
            prw1_sb = const.tile([G, NRAW, 128], F16)
            nc.gpsimd.dma_start(prw1_sb[:], prw1)

            modT = modp.tile([128, KO, T], F16)
            out_acc = oap.tile([128, KO, T], F32R)
            out_t = out.bitcast(F32R).rearrange("(ho p) t -> p ho t", p=128)

            h1b = {0: h1p.tile([128, FPB, T], F16, tag="h1", name="h1b0")}
            pfs = {}

            def ffn1_mm(fo, ko, last):
                # one K-tile of FFN1 tile fo; raw tiles keep the group open
                # for the later rank-16 correction
                src = xT if fo < NRAW else modT
                if ko == 0:
                    pfs[fo] = mmp.tile([128, T], F32, tag="mm", name=f"pf{fo}")
                nc.tensor.matmul(
                    pfs[fo][:], wts[fo][:, ko, :], src[:, ko, :],
                    start=(ko == 0), stop=(last and fo >= NRAW),
                )

            def ffn1_fixup(fo):
                # rank-16 correction: + prw1[fo]^T @ (0.5*effects^T), closing
                # the accumulation group, then evict through the exact gelu
                nc.tensor.matmul(
                    pfs[fo][:], prw1_sb[:, fo, :], effs_sb[:],
                    start=False, stop=True,
                )
                ffn1_gelu(fo)

            def ffn1_gelu(fo):
                nc.scalar.activation(
                    h1b[fo // FPB][:, fo % FPB, :], pfs[fo][:], AF.Gelu,
                    bias=fb1_sb[:, fo:fo + 1],
                )
                del pfs[fo]

            # ---- P1: causal-variable extraction interleaved with fo0 ----
            cv_ps = chp.tile([128, T], F32, tag="ch")
            for ko in range(KO):
                nc.tensor.matmul(
                    cv_ps[0:G, :], pe_sb[:, ko, :], xT[:, ko, :],
                    start=(ko == 0), stop=(ko == KO - 1),
                )
                ffn1_mm(0, ko, last=(ko == KO - 1))
            cvt_sb = smp.tile([G, T], F32R, tag="cv")
            nc.scalar.activation(cvt_sb[:], cv_ps[0:G, :], AF.Tanh,
                                 bias=zz[:])

            # ---- P2: mechanism chain hidden inside fo1's matmul stream ----
            for ko in range(4):
                ffn1_mm(1, ko, last=False)
            pi_ps = chp.tile([128, T], F32, tag="ch")
            nc.tensor.matmul(
                pi_ps[0:G, :], adj_sb[:], cvt_sb[:], start=True, stop=True
            )
            pit_sb = smp.tile([G, T], F32R, tag="pi")
            nc.vector.tensor_copy(pit_sb[:], pi_ps[0:G, :])
            for ko in range(4, 8):
                ffn1_mm(1, ko, last=False)
            h_ps = chp.tile([128, T], F32, tag="ch")
            nc.tensor.matmul(h_ps[:], w1a_sb[:], cvt_sb[:], start=True, stop=False)
            nc.tensor.matmul(h_ps[:], w1b_sb[:], pit_sb[:], start=False, stop=True)
            for ko in range(8, 12):
                ffn1_mm(1, ko, last=False)
            hm_sb = smp.tile([G * GH, T], F32R, tag="hm")
            nc.scalar.activation(hm_sb[:], h_ps[:], AF.Gelu, bias=b1f_sb[:])
            for ko in range(12, 14):
                ffn1_mm(1, ko, last=False)
            eff_ps = chp.tile([128, T], F32, tag="ch")
            nc.tensor.matmul(
                eff_ps[0:G, :], w2bd_sb[:], hm_sb[:], start=True, stop=True
            )
            for ko in range(14, 16):
                ffn1_mm(1, ko, last=True)
            # bias-add on DVE: keeps the ACT LUT on Gelu (no table reload)
            effs_sb = smp.tile([G, T], F16, tag="eff")
            nc.vector.tensor_scalar_add(effs_sb[:], eff_ps[0:G, :], b2s_sb[:])

            # ---- P3: fo2 raw; P4: corrections for fo0..2 ----
            wts[3] = w1p.tile([128, KO, 128], F16, tag="w1", name="wt3")
            nc.sync.dma_start(wts[3][:], fw1[3])
            for ko in range(KO):
                ffn1_mm(2, ko, last=(ko == KO - 1))
            for fo in range(3):
                ffn1_fixup(fo)

            # ---- P5: routing matmuls 1:1 with fo3 raw; modified -> modT ----
            for ho in range(KO):
                md = mdp.tile([128, T], F32, tag="md")
                nc.tensor.matmul(
                    md[:], pr_sb[:, ho * 128:(ho + 1) * 128], effs_sb[:],
                    start=True, stop=True,
                )
                ffn1_mm(3, ho, last=(ho == KO - 1))
                nc.vector.tensor_add(modT[:, ho, :], xT[:, ho, :], md[:])
            ffn1_fixup(3)

            # ---- P6: rest of block 0 on modified x ----
            for fo in range(NRAW, FPB):
                wts[fo] = w1p.tile([128, KO, 128], F16, tag="w1", name=f"wt{fo}")
                nc.sync.dma_start(wts[fo][:], fw1[fo])
                for ko in range(KO):
                    ffn1_mm(fo, ko, last=(ko == KO - 1))
                ffn1_gelu(fo)

            # ---- P7: FFN2 of block b interleaved with FFN1 of block b+1 ----
            for b in range(NBLK):
                if b + 1 < NBLK:
                    h1b[b + 1] = h1p.tile([128, FPB, T], F16, tag="h1", name=f"h1b{b+1}")
                for k in range(FPB):
                    if b + 1 < NBLK:
                        fo = (b + 1) * FPB + k
                        wts[fo] = w1p.tile([128, KO, 128], F16, tag="w1", name=f"wt{fo}")
                        nc.sync.dma_start(wts[fo][:], fw1[fo])
                        for ko in range(KO):
                            ffn1_mm(fo, ko, last=(ko == KO - 1))
                        ffn1_gelu(fo)
                    ho = k
                    w2t = w2p.tile([128, FPB, 128], F16, tag="w2")
                    nc.sync.dma_start(w2t[:], fw2[ho, b])
                    po = mmp.tile([128, T], F32, tag="mm")
                    for j in range(FPB):
                        nc.tensor.matmul(
                            po[:], w2t[:, j, :], h1b[b][:, j, :],
                            start=(j == 0), stop=(j == FPB - 1),
                        )
                    if b == 0:
                        nc.vector.tensor_scalar_add(
                            out_acc[:, ho, :], po[:], fb2_sb[:, ho:ho + 1]
                        )
                    else:
                        nc.vector.tensor_add(
                            out_acc[:, ho, :], out_acc[:, ho, :], po[:]
                        )
                    if b == NBLK - 1:
                        # store this H-tile feature-major; host transposes
                        nc.sync.dma_start(out_t[:, ho, :], out_acc[:, ho, :])

    nc.compile()
    return nc


def _prep(inputs):
    """Host-side restructuring of weights + sharding."""
    import ml_dtypes  # noqa: F401  (np.float16 used directly)

    hs = np.ascontiguousarray(np.asarray(inputs["hidden_states"], np.float32))
    W1 = np.asarray(inputs["W1"], np.float32)
    b1 = np.asarray(inputs["b1"], np.float32)
    W2 = np.asarray(inputs["W2"], np.float32)
    b2 = np.asarray(inputs["b2"], np.float32)

    w1a = np.ascontiguousarray(
        W1[:, :G, :].transpose(1, 0, 2).reshape(G, G * GH)
    )
    w1b = np.zeros((G, G * GH), np.float32)
    for m in range(G):
        w1b[m, m * GH:(m + 1) * GH] = W1[m, G, :]
    b1f = b1.reshape(G * GH, 1)
    w2bd = np.zeros((G * GH, G), np.float32)
    for m in range(G):
        w2bd[m * GH:(m + 1) * GH, m] = 0.5 * W2[m, :]
    b2s = (0.5 * b2).reshape(G, 1)

    pe = np.asarray(inputs["P_extract"], np.float32)
    # pe[h, g] -> [p, ko, g] with h = ko*128 + p
    pe_t = np.ascontiguousarray(
        pe.reshape(KO, 128, G).transpose(1, 0, 2)
    ).astype(np.float16)

    pr_f = np.asarray(inputs["P_route"], np.float32)
    fw1 = np.asarray(inputs["ffn_w1"], np.float32)
    # rank-16 correction weights for the raw leading tiles:
    # prw1[g, fo, f] = (P_route @ ffn_w1)[g, fo*128+f]
    prw1 = np.ascontiguousarray(
        (pr_f @ fw1[:, : NRAW * 128]).reshape(G, NRAW, 128)
    ).astype(np.float16)
    # fw1[ko*128+p, fo*128+f] -> [fo, p, ko, f]
    fw1_t = np.ascontiguousarray(
        fw1.reshape(KO, 128, FO, 128).transpose(2, 1, 0, 3)
    ).astype(np.float16)
    fw2 = np.asarray(inputs["ffn_w2"], np.float32)
    # fw2[(b*FPB+j)*128+p, ho*128+h] -> [ho, b, p, j, h]
    fw2_t = np.ascontiguousarray(
        fw2.reshape(NBLK, FPB, 128, KO, 128).transpose(3, 0, 2, 1, 4)
    ).astype(np.float16)

    common = {
        "pe": pe_t,
        "adj": np.ascontiguousarray(np.asarray(inputs["causal_adjacency"], np.float32)),
        "w1a": w1a,
        "w1b": w1b,
        "b1f": np.ascontiguousarray(b1f),
        "w2bd": w2bd,
        "b2s": np.ascontiguousarray(b2s),
        "pr": np.ascontiguousarray(pr_f).astype(np.float16),
        "prw1": prw1,
        "fw1": fw1_t,
        "fb1": np.ascontiguousarray(
            np.asarray(inputs["ffn_b1"], np.float32).reshape(FO, 128).T
        ),
        "fw2": fw2_t,
        "fb2": np.ascontiguousarray(
            np.asarray(inputs["ffn_b2"], np.float32).reshape(KO, 128).T
        ),
    }
    toks = hs.reshape(NTOK, H)
    in_maps = []
    for c in range(N_CORES):
        m = dict(common)
        m["xtd"] = np.ascontiguousarray(toks[c * T:(c + 1) * T].T).astype(
            np.float16
        )
        in_maps.append(m)
    return in_maps


def run(inputs, trace=False):
    """Returns (full output [B,S,H] fp32, BassKernelResults)."""
    if "nc" not in _CACHE:
        _CACHE["nc"] = _build()
    nc = _CACHE["nc"]
    in_maps = _prep(inputs)
    res = run_bass_kernel_spmd(
        nc, in_maps, core_ids=list(range(N_CORES)), trace=trace
    )
    full = np.empty((NTOK, H), np.float32)
    for c in range(N_CORES):
        full[c * T:(c + 1) * T] = res.results[c]["out"].T
    return full.reshape(B, S, H), res


def kernel(**inputs):
    full, _ = run(inputs, trace=False)
    return full


# revision 18
# speedup vs baseline: 1.0648x; 1.0101x over previous
"""Trainium2 Bass kernel for nn_CausalMoE.

Reference computation (B=2, S=2048, H=2048, G=16, GH=8, FFN=8192):
  cv        = tanh(hs @ P_extract)                        [N,G]   N = B*S = 4096
  pi        = cv @ A                                      [N,G]
  h[:,m,:]  = cv @ W1[m,:G,:] + pi[:,m,None]*W1[m,G,:] + b1[m]
  h         = gelu(h)  (exact erf gelu)                   [N,G,GH]
  effects   = sum_k h[:,m,k] W2[m,k] + b2[m]              [N,G]
  modified  = hs + 0.5 * effects @ P_route                [N,H]
  ffn_h     = gelu(modified @ ffn_w1 + ffn_b1)            [N,F]
  out       = ffn_h @ ffn_w2 + ffn_b2                     [N,H]

Strategy: pure data-parallel over the 8 NeuronCores (512 tokens/core),
weights replicated.  Everything is computed feature-major (activations
stored transposed, [feature, token]) so every matmul has its contraction
dim on partitions with weights as the stationary operand; the host
shards hs transposed and the gather transposes the output back, so the
kernel needs no on-chip transposes.  The FFN runs entirely in fp16
(e5m10): same 1-cycle/row PE speed class as bf16, FWL weight loads
(97 ns vs 187 ns for fp32 -- the f32r stream was weight-load bound at
227 ns/MM; fp16 hits the 216 ns N=512 issue floor), half the weight DMA
of f32r, and ~11-bit mantissas keep the max rel err ~6e-3.  The tiny
causal-mechanism chain stays f32r (its weights are folded into three
small matmuls via host-side restructuring).

The serial prologue is eliminated with a low-rank split of FFN layer 1:
modified @ ffn_w1 = hs @ ffn_w1 + effects @ (0.5 P_route @ ffn_w1),
where prw1 = P_route @ ffn_w1 [16, 8192] is precomputed on the host.
The first E=4 F-tiles of FFN1 run on raw hs (their rank-16 correction is
one K=16 matmul into the same PSUM accumulation before the gelu), so the
big matmul stream starts the moment the first x chunk + weight tile land
(~1 us) and the extraction/mechanism chain, routing matmuls and
modified-x evictions all hide inside it.  modified is written to a
separate tile (xT stays read-only) so the DVE evictions have no
ordering hazard against the raw-x matmuls.  FFN1 of block b+1 is
interleaved with FFN2 of block b to cover the last-gelu ACT latency at
each block boundary.  No PE idle gaps => the HAM clock gate stays at
2.4 GHz after the initial ~3.4 us ramp, with no junk keepalive matmuls.
"""
import sys

sys.path.insert(0, "/opt/trn_rl_repo")

import numpy as np

import concourse.bacc as bacc
import concourse.mybir as mybir
import concourse.tile as tile
from concourse.bass_utils import run_bass_kernel_spmd

F32 = mybir.dt.float32
F32R = mybir.dt.float32r
F16 = mybir.dt.float16
AF = mybir.ActivationFunctionType

B, S, H = 2, 2048, 2048
G, GH, F = 16, 8, 8192
N_CORES = 8
NTOK = B * S              # 4096 tokens total
T = NTOK // N_CORES       # 512 tokens per core
KO = H // 128             # 16 contraction tiles over H
FO = F // 128             # 64 F tiles
NBLK = 4                  # F blocks
FPB = FO // NBLK          # 16 F tiles per block
NRAW = 4                  # leading F-tiles computed on raw hs + rank-16 fixup

_CACHE = {}


def _build():
    nc = bacc.Bacc("TRN2", target_bir_lowering=False, debug=False)
    # host-side shard layout: xtd = hs_shard.T  [H, T] (feature-major, fp16)
    xtd = nc.dram_tensor("xtd", [H, T], F16, kind="ExternalInput").ap()
    pe = nc.dram_tensor("pe", [128, KO, G], F16, kind="ExternalInput").ap()
    adj = nc.dram_tensor("adj", [G, G], F32, kind="ExternalInput").ap()
    w1a = nc.dram_tensor("w1a", [G, G * GH], F32, kind="ExternalInput").ap()
    w1b = nc.dram_tensor("w1b", [G, G * GH], F32, kind="ExternalInput").ap()
    b1f = nc.dram_tensor("b1f", [G * GH, 1], F32, kind="ExternalInput").ap()
    w2bd = nc.dram_tensor("w2bd", [G * GH, G], F32, kind="ExternalInput").ap()
    b2s = nc.dram_tensor("b2s", [G, 1], F32, kind="ExternalInput").ap()
    pr = nc.dram_tensor("pr", [G, H], F16, kind="ExternalInput").ap()
    # prw1 = P_route @ ffn_w1, [g, fo, f]; correction weights for raw tiles
    prw1 = nc.dram_tensor("prw1", [G, NRAW, 128], F16, kind="ExternalInput").ap()
    # host-retiled: fw1t[fo, p, ko, f] = ffn_w1[ko*128+p, fo*128+f]  (fp16)
    fw1 = nc.dram_tensor("fw1", [FO, 128, KO, 128], F16, kind="ExternalInput").ap()
    fb1 = nc.dram_tensor("fb1", [128, FO], F32, kind="ExternalInput").ap()
    # host-retiled: fw2t[ho, b, p, j, h] = ffn_w2[(b*FPB+j)*128+p, ho*128+h]
    fw2 = nc.dram_tensor(
        "fw2", [KO, NBLK, 128, FPB, 128], F16, kind="ExternalInput"
    ).ap()
    fb2 = nc.dram_tensor("fb2", [128, KO], F32, kind="ExternalInput").ap()
    # output stays feature-major [H, T]; the host gather transposes
    out = nc.dram_tensor("out", [H, T], F32, kind="ExternalOutput").ap()

    with tile.TileContext(nc) as tc:
        with (
            tc.tile_pool(name="const", bufs=1) as const,
            tc.tile_pool(name="xt", bufs=1) as xtp,
            tc.tile_pool(name="mod", bufs=1) as modp,
            tc.tile_pool(name="h1", bufs=2) as h1p,
            tc.tile_pool(name="oacc", bufs=1) as oap,
            tc.tile_pool(name="w1", bufs=6) as w1p,
            tc.tile_pool(name="w2", bufs=5) as w2p,
            tc.tile_pool(name="sm", bufs=1) as smp,
            tc.tile_pool(name="mm", bufs=4, space="PSUM") as mmp,
            tc.tile_pool(name="md", bufs=2, space="PSUM") as mdp,
            tc.tile_pool(name="ch", bufs=2, space="PSUM") as chp,
        ):
            # explicit zero tile for activation biases: a float bias would
            # synthesize a const-AP pool whose TENSOR_LOAD sits in the
            # serialized kernel preamble (~2.7us)
            zz = const.tile([G, 1], F32)
            nc.gpsimd.memset(zz[:], 0.0)

            # warm the ACT Tanh+Gelu LUTs while DMAs land, so the ~1.3us
            # table loads are off the small-chain critical path
            act_warm = const.tile([1, 2], F32)
            nc.scalar.activation(act_warm[:, 0:1], zz[0:1, :], AF.Tanh,
                                 bias=zz[0:1, :])
            nc.scalar.activation(act_warm[:, 1:2], zz[0:1, :], AF.Gelu,
                                 bias=zz[0:1, :])

            # PE clock warm-up: the DMA queues take ~7.5us to deliver their
            # first payloads, during which the PE would otherwise idle cold
            # (HAM gates it to 1.2 GHz until ~3.4us of sustained activity).
            # Junk fp16 matmuls on a zeroed tile are free during that dead
            # zone and un-throttle the clock right as the real stream begins.
            scr = const.tile([128, T], F16)
            nc.gpsimd.memset(scr[:], 0.0)
            jp = mmp.tile([128, T], F32, tag="mm")
            for _ in range(8):
                nc.tensor.matmul(
                    jp[:], scr[:, 0:128], scr[:], start=True, stop=True
                )

            # extraction weights first on the fast sync queue (first matmul
            # needs them); small consts ride the gpsimd queue
            pe_sb = const.tile([128, KO, G], F16)
            nc.sync.dma_start(pe_sb[:], pe)
            adj_sb = const.tile([G, G], F32R)
            nc.gpsimd.dma_start(adj_sb[:], adj.bitcast(F32R))
            w1a_sb = const.tile([G, G * GH], F32R)
            nc.gpsimd.dma_start(w1a_sb[:], w1a.bitcast(F32R))
            w1b_sb = const.tile([G, G * GH], F32R)
            nc.gpsimd.dma_start(w1b_sb[:], w1b.bitcast(F32R))
            b1f_sb = const.tile([G * GH, 1], F32)
            nc.gpsimd.dma_start(b1f_sb[:], b1f)
            w2bd_sb = const.tile([G * GH, G], F32R)
            nc.gpsimd.dma_start(w2bd_sb[:], w2bd.bitcast(F32R))
            b2s_sb = const.tile([G, 1], F32)
            nc.gpsimd.dma_start(b2s_sb[:], b2s)
            fb1_sb = const.tile([128, FO], F32)
            nc.gpsimd.dma_start(fb1_sb[:], fb1)
            fb2_sb = const.tile([128, KO], F32)
            nc.gpsimd.dma_start(fb2_sb[:], fb2)

            # ---- feature-major xT [128, KO, T], split across two queues ----
            # (read-only for the whole kernel: raw tiles + routing read it;
            # modified goes to a separate tile, so no version hazards.
            # The scalar engine's DMA queue is idle until the first gelu at
            # ~18us, so the odd chunks + routing weights ride it for free;
            # the slow gpsimd queue only carries tiny consts.)
            xT = xtp.tile([128, KO, T], F16)
            xtd_t = xtd.rearrange("(ko p) t -> p ko t", p=128)
            nc.sync.dma_start(xT[:, 0:2, :], xtd_t[:, 0:2, :])
            nc.scalar.dma_start(xT[:, 2:4, :], xtd_t[:, 2:4, :])
            nc.scalar.dma_start(xT[:, 6:8, :], xtd_t[:, 6:8, :])
            nc.scalar.dma_start(xT[:, 10:12, :], xtd_t[:, 10:12, :])
            nc.scalar.dma_start(xT[:, 14:16, :], xtd_t[:, 14:16, :])

            # first raw-tile weights interleaved with the even x chunks
            wts = {}
            wts[0] = w1p.tile([128, KO, 128], F16, tag="w1", name="wt0")
            nc.sync.dma_start(wts[0][:], fw1[0])
            nc.sync.dma_start(xT[:, 4:6, :], xtd_t[:, 4:6, :])
            wts[1] = w1p.tile([128, KO, 128], F16, tag="w1", name="wt1")
            nc.sync.dma_start(wts[1][:], fw1[1])
            nc.sync.dma_start(xT[:, 8:10, :], xtd_t[:, 8:10, :])
            nc.sync.dma_start(xT[:, 12:14, :], xtd_t[:, 12:14, :])
            wts[2] = w1p.tile([128, KO, 128], F16, tag="w1", name="wt2")
            nc.sync.dma_start(wts[2][:], fw1[2])

            # routing / correction weights (needed ~15us+) on the scalar queue
            pr_sb = const.tile([G, H], F16)
            nc.scalar.dma_start(pr_sb[:], pr)
            prw1_sb = const.tile([G, NRAW, 128], F16)
            nc.scalar.dma_start(prw1_sb[:], prw1)

            modT = modp.tile([128, KO, T], F16)
            out_acc = oap.tile([128, KO, T], F32R)
            out_t = out.bitcast(F32R).rearrange("(ho p) t -> p ho t", p=128)

            h1b = {0: h1p.tile([128, FPB, T], F16, tag="h1", name="h1b0")}
            pfs = {}

            def ffn1_mm(fo, ko, last):
                # one K-tile of FFN1 tile fo; raw tiles keep the group open
                # for the later rank-16 correction
                src = xT if fo < NRAW else modT
                if ko == 0:
                    pfs[fo] = mmp.tile([128, T], F32, tag="mm", name=f"pf{fo}")
                nc.tensor.matmul(
                    pfs[fo][:], wts[fo][:, ko, :], src[:, ko, :],
                    start=(ko == 0), stop=(last and fo >= NRAW),
                )

            def ffn1_fixup(fo):
                # rank-16 correction: + prw1[fo]^T @ (0.5*effects^T), closing
                # the accumulation group, then evict through the exact gelu
                nc.tensor.matmul(
                    pfs[fo][:], prw1_sb[:, fo, :], effs_sb[:],
                    start=False, stop=True,
                )
                ffn1_gelu(fo)

            def ffn1_gelu(fo):
                nc.scalar.activation(
                    h1b[fo // FPB][:, fo % FPB, :], pfs[fo][:], AF.Gelu,
                    bias=fb1_sb[:, fo:fo + 1],
                )
                del pfs[fo]

            # ---- P1: causal-variable extraction interleaved with fo0 ----
            cv_ps = chp.tile([128, T], F32, tag="ch")
            for ko in range(KO):
                nc.tensor.matmul(
                    cv_ps[0:G, :], pe_sb[:, ko, :], xT[:, ko, :],
                    start=(ko == 0), stop=(ko == KO - 1),
                )
                ffn1_mm(0, ko, last=(ko == KO - 1))
            cvt_sb = smp.tile([G, T], F32R, tag="cv")
            nc.scalar.activation(cvt_sb[:], cv_ps[0:G, :], AF.Tanh,
                                 bias=zz[:])

            # ---- P2: mechanism chain hidden inside fo1's matmul stream ----
            for ko in range(4):
                ffn1_mm(1, ko, last=False)
            pi_ps = chp.tile([128, T], F32, tag="ch")
            nc.tensor.matmul(
                pi_ps[0:G, :], adj_sb[:], cvt_sb[:], start=True, stop=True
            )
            pit_sb = smp.tile([G, T], F32R, tag="pi")
            nc.vector.tensor_copy(pit_sb[:], pi_ps[0:G, :])
            for ko in range(4, 8):
                ffn1_mm(1, ko, last=False)
            h_ps = chp.tile([128, T], F32, tag="ch")
            nc.tensor.matmul(h_ps[:], w1a_sb[:], cvt_sb[:], start=True, stop=False)
            nc.tensor.matmul(h_ps[:], w1b_sb[:], pit_sb[:], start=False, stop=True)
            for ko in range(8, 12):
                ffn1_mm(1, ko, last=False)
            hm_sb = smp.tile([G * GH, T], F32R, tag="hm")
            nc.scalar.activation(hm_sb[:], h_ps[:], AF.Gelu, bias=b1f_sb[:])
            for ko in range(12, 14):
                ffn1_mm(1, ko, last=False)
            eff_ps = chp.tile([128, T], F32, tag="ch")
            nc.tensor.matmul(
                eff_ps[0:G, :], w2bd_sb[:], hm_sb[:], start=True, stop=True
            )
            for ko in range(14, 16):
                ffn1_mm(1, ko, last=True)
            # bias-add on DVE: keeps the ACT LUT on Gelu (no table reload)
            effs_sb = smp.tile([G, T], F16, tag="eff")
            nc.vector.tensor_scalar_add(effs_sb[:], eff_ps[0:G, :], b2s_sb[:])

            # ---- P3: fo2 raw; P4: corrections for fo0..2 ----
            wts[3] = w1p.tile([128, KO, 128], F16, tag="w1", name="wt3")
            nc.sync.dma_start(wts[3][:], fw1[3])
            for ko in range(KO):
                ffn1_mm(2, ko, last=(ko == KO - 1))
            for fo in range(3):
                ffn1_fixup(fo)

            # ---- P5: routing matmuls 1:1 with fo3 raw; modified -> modT ----
            for ho in range(KO):
                md = mdp.tile([128, T], F32, tag="md")
                nc.tensor.matmul(
                    md[:], pr_sb[:, ho * 128:(ho + 1) * 128], effs_sb[:],
                    start=True, stop=True,
                )
                ffn1_mm(3, ho, last=(ho == KO - 1))
                nc.vector.tensor_add(modT[:, ho, :], xT[:, ho, :], md[:])
            ffn1_fixup(3)

            # ---- P6: rest of block 0 on modified x ----
            for fo in range(NRAW, FPB):
                wts[fo] = w1p.tile([128, KO, 128], F16, tag="w1", name=f"wt{fo}")
                nc.sync.dma_start(wts[fo][:], fw1[fo])
                for ko in range(KO):
                    ffn1_mm(fo, ko, last=(ko == KO - 1))
                ffn1_gelu(fo)

            # ---- P7: FFN2 of block b interleaved with FFN1 of block b+1 ----
            for b in range(NBLK):
                if b + 1 < NBLK:
                    h1b[b + 1] = h1p.tile([128, FPB, T], F16, tag="h1", name=f"h1b{b+1}")
                for k in range(FPB):
                    if b + 1 < NBLK:
                        fo = (b + 1) * FPB + k
                        wts[fo] = w1p.tile([128, KO, 128], F16, tag="w1", name=f"wt{fo}")
                        nc.sync.dma_start(wts[fo][:], fw1[fo])
                        for ko in range(KO):
                            ffn1_mm(fo, ko, last=(ko == KO - 1))
                        ffn1_gelu(fo)
                    ho = k
                    w2t = w2p.tile([128, FPB, 128], F16, tag="w2")
                    nc.sync.dma_start(w2t[:], fw2[ho, b])
                    po = mmp.tile([128, T], F32, tag="mm")
                    for j in range(FPB):
                        nc.tensor.matmul(
                            po[:], w2t[:, j, :], h1b[b][:, j, :],
                            start=(j == 0), stop=(j == FPB - 1),
                        )
                    if b == 0:
                        nc.vector.tensor_scalar_add(
                            out_acc[:, ho, :], po[:], fb2_sb[:, ho:ho + 1]
                        )
                    else:
                        nc.vector.tensor_add(
                            out_acc[:, ho, :], out_acc[:, ho, :], po[:]
                        )
                    if b == NBLK - 1:
                        # store this H-tile feature-major; host transposes
                        nc.sync.dma_start(out_t[:, ho, :], out_acc[:, ho, :])

    nc.compile()
    return nc


def _prep(inputs):
    """Host-side restructuring of weights + sharding."""
    import ml_dtypes  # noqa: F401  (np.float16 used directly)

    hs = np.ascontiguousarray(np.asarray(inputs["hidden_states"], np.float32))
    W1 = np.asarray(inputs["W1"], np.float32)
    b1 = np.asarray(inputs["b1"], np.float32)
    W2 = np.asarray(inputs["W2"], np.float32)
    b2 = np.asarray(inputs["b2"], np.float32)

    w1a = np.ascontiguousarray(
        W1[:, :G, :].transpose(1, 0, 2).reshape(G, G * GH)
    )
    w1b = np.zeros((G, G * GH), np.float32)
    for m in range(G):
        w1b[m, m * GH:(m + 1) * GH] = W1[m, G, :]
    b1f = b1.reshape(G * GH, 1)
    w2bd = np.zeros((G * GH, G), np.float32)
    for m in range(G):
        w2bd[m * GH:(m + 1) * GH, m] = 0.5 * W2[m, :]
    b2s = (0.5 * b2).reshape(G, 1)

    pe = np.asarray(inputs["P_extract"], np.float32)
    # pe[h, g] -> [p, ko, g] with h = ko*128 + p
    pe_t = np.ascontiguousarray(
        pe.reshape(KO, 128, G).transpose(1, 0, 2)
    ).astype(np.float16)

    pr_f = np.asarray(inputs["P_route"], np.float32)
    fw1 = np.asarray(inputs["ffn_w1"], np.float32)
    # rank-16 correction weights for the raw leading tiles:
    # prw1[g, fo, f] = (P_route @ ffn_w1)[g, fo*128+f]
    prw1 = np.ascontiguousarray(
        (pr_f @ fw1[:, : NRAW * 128]).reshape(G, NRAW, 128)
    ).astype(np.float16)
    # fw1[ko*128+p, fo*128+f] -> [fo, p, ko, f]
    fw1_t = np.ascontiguousarray(
        fw1.reshape(KO, 128, FO, 128).transpose(2, 1, 0, 3)
    ).astype(np.float16)
    fw2 = np.asarray(inputs["ffn_w2"], np.float32)
    # fw2[(b*FPB+j)*128+p, ho*128+h] -> [ho, b, p, j, h]
    fw2_t = np.ascontiguousarray(
        fw2.reshape(NBLK, FPB, 128, KO, 128).transpose(3, 0, 2, 1, 4)
    ).astype(np.float16)

    common = {
        "pe": pe_t,
        "adj": np.ascontiguousarray(np.asarray(inputs["causal_adjacency"], np.float32)),
        "w1a": w1a,
        "w1b": w1b,
        "b1f": np.ascontiguousarray(b1f),
        "w2bd": w2bd,
        "b2s": np.ascontiguousarray(b2s),
        "pr": np.ascontiguousarray(pr_f).astype(np.float16),
        "prw1": prw1,
        "fw1": fw1_t,
        "fb1": np.ascontiguousarray(
            np.asarray(inputs["ffn_b1"], np.float32).reshape(FO, 128).T
        ),
        "fw2": fw2_t,
        "fb2": np.ascontiguousarray(
            np.asarray(inputs["ffn_b2"], np.float32).reshape(KO, 128).T
        ),
    }
    toks = hs.reshape(NTOK, H)
    in_maps = []
    for c in range(N_CORES):
        m = dict(common)
        m["xtd"] = np.ascontiguousarray(toks[c * T:(c + 1) * T].T).astype(
            np.float16
        )
        in_maps.append(m)
    return in_maps


def run(inputs, trace=False):
    """Returns (full output [B,S,H] fp32, BassKernelResults)."""
    if "nc" not in _CACHE:
        _CACHE["nc"] = _build()
    nc = _CACHE["nc"]
    in_maps = _prep(inputs)
    res = run_bass_kernel_spmd(
        nc, in_maps, core_ids=list(range(N_CORES)), trace=trace
    )
    full = np.empty((NTOK, H), np.float32)
    for c in range(N_CORES):
        full[c * T:(c + 1) * T] = res.results[c]["out"].T
    return full.reshape(B, S, H), res


def kernel(**inputs):
    full, _ = run(inputs, trace=False)
    return full


# revision 22
# speedup vs baseline: 1.0727x; 1.0074x over previous
"""Trainium2 Bass kernel for nn_CausalMoE.

Reference computation (B=2, S=2048, H=2048, G=16, GH=8, FFN=8192):
  cv        = tanh(hs @ P_extract)                        [N,G]   N = B*S = 4096
  pi        = cv @ A                                      [N,G]
  h[:,m,:]  = cv @ W1[m,:G,:] + pi[:,m,None]*W1[m,G,:] + b1[m]
  h         = gelu(h)  (exact erf gelu)                   [N,G,GH]
  effects   = sum_k h[:,m,k] W2[m,k] + b2[m]              [N,G]
  modified  = hs + 0.5 * effects @ P_route                [N,H]
  ffn_h     = gelu(modified @ ffn_w1 + ffn_b1)            [N,F]
  out       = ffn_h @ ffn_w2 + ffn_b2                     [N,H]

Strategy: pure data-parallel over the 8 NeuronCores (512 tokens/core),
weights replicated.  Everything is computed feature-major (activations
stored transposed, [feature, token]) so every matmul has its contraction
dim on partitions with weights as the stationary operand; the host
shards hs transposed and the gather transposes the output back, so the
kernel needs no on-chip transposes.  The FFN runs entirely in fp16
(e5m10): same 1-cycle/row PE speed class as bf16, FWL weight loads
(97 ns vs 187 ns for fp32 -- the f32r stream was weight-load bound at
227 ns/MM; fp16 hits the 216 ns N=512 issue floor), half the weight DMA
of f32r, and ~11-bit mantissas keep the max rel err ~6e-3.  The tiny
causal-mechanism chain stays f32r (its weights are folded into three
small matmuls via host-side restructuring).

The serial prologue is eliminated with a low-rank split of FFN layer 1:
modified @ ffn_w1 = hs @ ffn_w1 + effects @ (0.5 P_route @ ffn_w1),
where prw1 = P_route @ ffn_w1 [16, 8192] is precomputed on the host.
The first E=4 F-tiles of FFN1 run on raw hs (their rank-16 correction is
one K=16 matmul into the same PSUM accumulation before the gelu), so the
big matmul stream starts the moment the first x chunk + weight tile land
(~1 us) and the extraction/mechanism chain, routing matmuls and
modified-x evictions all hide inside it.  modified is written to a
separate tile (xT stays read-only) so the DVE evictions have no
ordering hazard against the raw-x matmuls.  FFN1 of block b+1 is
interleaved with FFN2 of block b to cover the last-gelu ACT latency at
each block boundary.  No PE idle gaps => the HAM clock gate stays at
2.4 GHz after the initial ~3.4 us ramp, with no junk keepalive matmuls.
"""
import sys

sys.path.insert(0, "/opt/trn_rl_repo")

import numpy as np

import concourse.bacc as bacc
import concourse.mybir as mybir
import concourse.tile as tile
from concourse.bass_utils import run_bass_kernel_spmd

F32 = mybir.dt.float32
F32R = mybir.dt.float32r
F16 = mybir.dt.float16
AF = mybir.ActivationFunctionType

B, S, H = 2, 2048, 2048
G, GH, F = 16, 8, 8192
N_CORES = 8
NTOK = B * S              # 4096 tokens total
T = NTOK // N_CORES       # 512 tokens per core
KO = H // 128             # 16 contraction tiles over H
FO = F // 128             # 64 F tiles
NBLK = 4                  # F blocks
FPB = FO // NBLK          # 16 F tiles per block
NRAW = 5                  # leading F-tiles computed on raw hs + rank-16 fixup

_CACHE = {}


def _build():
    nc = bacc.Bacc("TRN2", target_bir_lowering=False, debug=False)
    # host-side shard layout: xtd = hs_shard.T  [H, T] (feature-major, fp16)
    xtd = nc.dram_tensor("xtd", [H, T], F16, kind="ExternalInput").ap()
    pe = nc.dram_tensor("pe", [128, KO, G], F16, kind="ExternalInput").ap()
    adj = nc.dram_tensor("adj", [G, G], F32, kind="ExternalInput").ap()
    w1a = nc.dram_tensor("w1a", [G, G * GH], F32, kind="ExternalInput").ap()
    w1b = nc.dram_tensor("w1b", [G, G * GH], F32, kind="ExternalInput").ap()
    b1f = nc.dram_tensor("b1f", [G * GH, 1], F32, kind="ExternalInput").ap()
    w2bd = nc.dram_tensor("w2bd", [G * GH, G], F32, kind="ExternalInput").ap()
    b2s = nc.dram_tensor("b2s", [G, 1], F32, kind="ExternalInput").ap()
    pr = nc.dram_tensor("pr", [G, H], F16, kind="ExternalInput").ap()
    # prw1 = P_route @ ffn_w1, [g, fo, f]; correction weights for raw tiles
    prw1 = nc.dram_tensor("prw1", [G, NRAW, 128], F16, kind="ExternalInput").ap()
    # host-retiled: fw1t[fo, p, ko, f] = ffn_w1[ko*128+p, fo*128+f]  (fp16)
    fw1 = nc.dram_tensor("fw1", [FO, 128, KO, 128], F16, kind="ExternalInput").ap()
    fb1 = nc.dram_tensor("fb1", [128, FO], F32, kind="ExternalInput").ap()
    # host-retiled: fw2t[ho, b, p, j, h] = ffn_w2[(b*FPB+j)*128+p, ho*128+h]
    fw2 = nc.dram_tensor(
        "fw2", [KO, NBLK, 128, FPB, 128], F16, kind="ExternalInput"
    ).ap()
    fb2 = nc.dram_tensor("fb2", [128, KO], F32, kind="ExternalInput").ap()
    # output stays feature-major [H, T]; the host gather transposes
    out = nc.dram_tensor("out", [H, T], F32, kind="ExternalOutput").ap()

    with tile.TileContext(nc) as tc:
        with (
            tc.tile_pool(name="const", bufs=1) as const,
            tc.tile_pool(name="xt", bufs=1) as xtp,
            tc.tile_pool(name="mod", bufs=1) as modp,
            tc.tile_pool(name="h1", bufs=2) as h1p,
            tc.tile_pool(name="oacc", bufs=1) as oap,
            tc.tile_pool(name="w1", bufs=6) as w1p,
            tc.tile_pool(name="w2", bufs=5) as w2p,
            tc.tile_pool(name="sm", bufs=1) as smp,
            tc.tile_pool(name="mm", bufs=4, space="PSUM") as mmp,
            tc.tile_pool(name="md", bufs=2, space="PSUM") as mdp,
            tc.tile_pool(name="ch", bufs=2, space="PSUM") as chp,
        ):
            # explicit zero tile for activation biases: a float bias would
            # synthesize a const-AP pool whose TENSOR_LOAD sits in the
            # serialized kernel preamble (~2.7us)
            zz = const.tile([G, 1], F32)
            nc.gpsimd.memset(zz[:], 0.0)

            # warm the ACT Tanh+Gelu LUTs while DMAs land, so the ~1.3us
            # table loads are off the small-chain critical path
            act_warm = const.tile([1, 2], F32)
            nc.scalar.activation(act_warm[:, 0:1], zz[0:1, :], AF.Tanh,
                                 bias=zz[0:1, :])
            nc.scalar.activation(act_warm[:, 1:2], zz[0:1, :], AF.Gelu,
                                 bias=zz[0:1, :])

            # PE clock warm-up: the DMA queues take ~8.5us to deliver their
            # first payloads, during which the PE would otherwise idle cold
            # (HAM gates it to 1.2 GHz until ~3.4us of sustained activity).
            # A couple of junk fp16 matmuls on a zeroed tile bridge the gap
            # between PE-queue start and the first x chunk landing.
            scr = const.tile([128, T], F16)
            nc.gpsimd.memset(scr[:], 0.0)
            jp = mmp.tile([128, T], F32, tag="mm")
            for _ in range(2):
                nc.tensor.matmul(
                    jp[:], scr[:, 0:128], scr[:], start=True, stop=True
                )

            # small consts on the gpsimd queue (pe first -- the first real
            # matmul needs it)
            pe_sb = const.tile([128, KO, G], F16)
            nc.gpsimd.dma_start(pe_sb[:], pe)
            adj_sb = const.tile([G, G], F32R)
            nc.gpsimd.dma_start(adj_sb[:], adj.bitcast(F32R))
            w1a_sb = const.tile([G, G * GH], F32R)
            nc.gpsimd.dma_start(w1a_sb[:], w1a.bitcast(F32R))
            w1b_sb = const.tile([G, G * GH], F32R)
            nc.gpsimd.dma_start(w1b_sb[:], w1b.bitcast(F32R))
            b1f_sb = const.tile([G * GH, 1], F32)
            nc.gpsimd.dma_start(b1f_sb[:], b1f)
            w2bd_sb = const.tile([G * GH, G], F32R)
            nc.gpsimd.dma_start(w2bd_sb[:], w2bd.bitcast(F32R))
            b2s_sb = const.tile([G, 1], F32)
            nc.gpsimd.dma_start(b2s_sb[:], b2s)
            fb1_sb = const.tile([128, FO], F32)
            nc.gpsimd.dma_start(fb1_sb[:], fb1)
            fb2_sb = const.tile([128, KO], F32)
            nc.gpsimd.dma_start(fb2_sb[:], fb2)

            # ---- feature-major xT [128, KO, T] ----
            # Read-only for the whole kernel: raw tiles + routing read it;
            # modified goes to a separate tile, so no version hazards.
            # The early phase is HBM-bandwidth-bound (~3.5 MiB of x + leading
            # weights at ~300 GB/s), so ALL critical payload rides ONE fast
            # queue in exact consumption order; the tiny routing weights go
            # on the otherwise-idle scalar queue and consts on gpsimd.
            xT = xtp.tile([128, KO, T], F16)
            xtd_t = xtd.rearrange("(ko p) t -> p ko t", p=128)
            wts = {}

            def wt_dma(fo):
                wts[fo] = w1p.tile(
                    [128, KO, 128], F16, tag="w1", name=f"wt{fo}"
                )
                nc.sync.dma_start(wts[fo][:], fw1[fo])

            nc.sync.dma_start(xT[:, 0:2, :], xtd_t[:, 0:2, :])
            wt_dma(0)
            nc.sync.dma_start(xT[:, 2:4, :], xtd_t[:, 2:4, :])
            wt_dma(1)
            nc.sync.dma_start(xT[:, 4:6, :], xtd_t[:, 4:6, :])
            nc.sync.dma_start(xT[:, 6:8, :], xtd_t[:, 6:8, :])
            wt_dma(2)
            nc.sync.dma_start(xT[:, 8:10, :], xtd_t[:, 8:10, :])
            nc.sync.dma_start(xT[:, 10:12, :], xtd_t[:, 10:12, :])
            nc.sync.dma_start(xT[:, 12:14, :], xtd_t[:, 12:14, :])
            nc.sync.dma_start(xT[:, 14:16, :], xtd_t[:, 14:16, :])
            wt_dma(3)
            wt_dma(4)

            # routing / correction weights (needed ~25us+) on the scalar queue
            pr_sb = const.tile([G, H], F16)
            nc.scalar.dma_start(pr_sb[:], pr)
            prw1_sb = const.tile([G, NRAW, 128], F16)
            nc.scalar.dma_start(prw1_sb[:], prw1)

            modT = modp.tile([128, KO, T], F16)
            out_acc = oap.tile([128, KO, T], F32R)
            out_t = out.bitcast(F32R).rearrange("(ho p) t -> p ho t", p=128)

            h1b = {0: h1p.tile([128, FPB, T], F16, tag="h1", name="h1b0")}
            pfs = {}

            def ffn1_mm(fo, ko, last):
                # one K-tile of FFN1 tile fo; raw tiles keep the group open
                # for the later rank-16 correction
                src = xT if fo < NRAW else modT
                if ko == 0:
                    pfs[fo] = mmp.tile([128, T], F32, tag="mm", name=f"pf{fo}")
                nc.tensor.matmul(
                    pfs[fo][:], wts[fo][:, ko, :], src[:, ko, :],
                    start=(ko == 0), stop=(last and fo >= NRAW),
                )

            def ffn1_fixup(fo):
                # rank-16 correction: + prw1[fo]^T @ (0.5*effects^T), closing
                # the accumulation group, then evict through the exact gelu
                nc.tensor.matmul(
                    pfs[fo][:], prw1_sb[:, fo, :], effs_sb[:],
                    start=False, stop=True,
                )
                ffn1_gelu(fo)

            def ffn1_gelu(fo):
                nc.scalar.activation(
                    h1b[fo // FPB][:, fo % FPB, :], pfs[fo][:], AF.Gelu,
                    bias=fb1_sb[:, fo:fo + 1],
                )
                del pfs[fo]

            # ---- P1: wavefront over arriving x chunks ----
            # Chunk c (kos 2c, 2c+1) unlocks: extraction of those kos, plus
            # fo0/fo1/fo2 matmuls lagged 1/2/3 chunks behind, so the PE
            # always has landed data queued and consumption (~8 MMs/chunk)
            # matches the DMA arrival rate -- no starvation stall, and the
            # HAM clock gate stays engaged once warm.
            cv_ps = chp.tile([128, T], F32, tag="ch")
            for c in range(8):
                for ko in (2 * c, 2 * c + 1):
                    nc.tensor.matmul(
                        cv_ps[0:G, :], pe_sb[:, ko, :], xT[:, ko, :],
                        start=(ko == 0), stop=(ko == KO - 1),
                    )
                for lag, fo in ((1, 0), (2, 1), (3, 2)):
                    cb = c - lag
                    if cb >= 0:
                        ffn1_mm(fo, 2 * cb, last=False)
                        ffn1_mm(fo, 2 * cb + 1, last=False)
            cvt_sb = smp.tile([G, T], F32R, tag="cv")
            nc.scalar.activation(cvt_sb[:], cv_ps[0:G, :], AF.Tanh,
                                 bias=zz[:])

            # ---- P2: wavefront tails with the mechanism chain sprinkled in
            # (each chain matmul sits behind enough FFN1 work to hide the
            # ACT/DVE latency of the step feeding it)
            ffn1_mm(0, 14, last=False)
            ffn1_mm(0, 15, last=False)
            pi_ps = chp.tile([128, T], F32, tag="ch")
            nc.tensor.matmul(
                pi_ps[0:G, :], adj_sb[:], cvt_sb[:], start=True, stop=True
            )
            pit_sb = smp.tile([G, T], F32R, tag="pi")
            nc.vector.tensor_copy(pit_sb[:], pi_ps[0:G, :])
            ffn1_mm(1, 12, last=False)
            ffn1_mm(1, 13, last=False)
            h_ps = chp.tile([128, T], F32, tag="ch")
            nc.tensor.matmul(h_ps[:], w1a_sb[:], cvt_sb[:], start=True, stop=False)
            nc.tensor.matmul(h_ps[:], w1b_sb[:], pit_sb[:], start=False, stop=True)
            ffn1_mm(1, 14, last=False)
            ffn1_mm(1, 15, last=False)
            hm_sb = smp.tile([G * GH, T], F32R, tag="hm")
            nc.scalar.activation(hm_sb[:], h_ps[:], AF.Gelu, bias=b1f_sb[:])
            ffn1_mm(2, 10, last=False)
            ffn1_mm(2, 11, last=False)
            eff_ps = chp.tile([128, T], F32, tag="ch")
            nc.tensor.matmul(
                eff_ps[0:G, :], w2bd_sb[:], hm_sb[:], start=True, stop=True
            )
            for ko in range(12, 16):
                ffn1_mm(2, ko, last=False)
            # bias-add on DVE: keeps the ACT LUT on Gelu (no table reload)
            effs_sb = smp.tile([G, T], F16, tag="eff")
            nc.vector.tensor_scalar_add(effs_sb[:], eff_ps[0:G, :], b2s_sb[:])

            # ---- P4: corrections for fo0..2 ----
            for fo in range(3):
                ffn1_fixup(fo)

            # ---- P5: routing matmuls 1:2 with fo3/fo4 raw (the modified-x
            # DVE evictions run at ~550ns each vs 432ns for a 1:1 interleave,
            # so two raw streams keep the PE ahead of the eviction rate) ----
            for ho in range(KO):
                md = mdp.tile([128, T], F32, tag="md")
                nc.tensor.matmul(
                    md[:], pr_sb[:, ho * 128:(ho + 1) * 128], effs_sb[:],
                    start=True, stop=True,
                )
                ffn1_mm(3, ho, last=False)
                ffn1_mm(4, ho, last=False)
                nc.vector.tensor_add(modT[:, ho, :], xT[:, ho, :], md[:])
            ffn1_fixup(3)
            ffn1_fixup(4)

            # ---- P6: rest of block 0 on modified x ----
            for fo in range(NRAW, FPB):
                wt_dma(fo)
                for ko in range(KO):
                    ffn1_mm(fo, ko, last=(ko == KO - 1))
                ffn1_gelu(fo)

            # ---- P7: FFN2 of block b interleaved with FFN1 of block b+1 ----
            for b in range(NBLK):
                if b + 1 < NBLK:
                    h1b[b + 1] = h1p.tile([128, FPB, T], F16, tag="h1", name=f"h1b{b+1}")
                for k in range(FPB):
                    if b + 1 < NBLK:
                        fo = (b + 1) * FPB + k
                        wts[fo] = w1p.tile([128, KO, 128], F16, tag="w1", name=f"wt{fo}")
                        nc.sync.dma_start(wts[fo][:], fw1[fo])
                        for ko in range(KO):
                            ffn1_mm(fo, ko, last=(ko == KO - 1))
                        ffn1_gelu(fo)
                    ho = k
                    w2t = w2p.tile([128, FPB, 128], F16, tag="w2")
                    nc.sync.dma_start(w2t[:], fw2[ho, b])
                    po = mmp.tile([128, T], F32, tag="mm")
                    for j in range(FPB):
                        nc.tensor.matmul(
                            po[:], w2t[:, j, :], h1b[b][:, j, :],
                            start=(j == 0), stop=(j == FPB - 1),
                        )
                    if b == 0:
                        nc.vector.tensor_scalar_add(
                            out_acc[:, ho, :], po[:], fb2_sb[:, ho:ho + 1]
                        )
                    else:
                        nc.vector.tensor_add(
                            out_acc[:, ho, :], out_acc[:, ho, :], po[:]
                        )
                    if b == NBLK - 1:
                        # store this H-tile feature-major; host transposes
                        nc.sync.dma_start(out_t[:, ho, :], out_acc[:, ho, :])

    nc.compile()
    return nc


def _prep(inputs):
    """Host-side restructuring of weights + sharding."""
    import ml_dtypes  # noqa: F401  (np.float16 used directly)

    hs = np.ascontiguousarray(np.asarray(inputs["hidden_states"], np.float32))
    W1 = np.asarray(inputs["W1"], np.float32)
    b1 = np.asarray(inputs["b1"], np.float32)
    W2 = np.asarray(inputs["W2"], np.float32)
    b2 = np.asarray(inputs["b2"], np.float32)

    w1a = np.ascontiguousarray(
        W1[:, :G, :].transpose(1, 0, 2).reshape(G, G * GH)
    )
    w1b = np.zeros((G, G * GH), np.float32)
    for m in range(G):
        w1b[m, m * GH:(m + 1) * GH] = W1[m, G, :]
    b1f = b1.reshape(G * GH, 1)
    w2bd = np.zeros((G * GH, G), np.float32)
    for m in range(G):
        w2bd[m * GH:(m + 1) * GH, m] = 0.5 * W2[m, :]
    b2s = (0.5 * b2).reshape(G, 1)

    pe = np.asarray(inputs["P_extract"], np.float32)
    # pe[h, g] -> [p, ko, g] with h = ko*128 + p
    pe_t = np.ascontiguousarray(
        pe.reshape(KO, 128, G).transpose(1, 0, 2)
    ).astype(np.float16)

    pr_f = np.asarray(inputs["P_route"], np.float32)
    fw1 = np.asarray(inputs["ffn_w1"], np.float32)
    # rank-16 correction weights for the raw leading tiles:
    # prw1[g, fo, f] = (P_route @ ffn_w1)[g, fo*128+f]
    prw1 = np.ascontiguousarray(
        (pr_f @ fw1[:, : NRAW * 128]).reshape(G, NRAW, 128)
    ).astype(np.float16)
    # fw1[ko*128+p, fo*128+f] -> [fo, p, ko, f]
    fw1_t = np.ascontiguousarray(
        fw1.reshape(KO, 128, FO, 128).transpose(2, 1, 0, 3)
    ).astype(np.float16)
    fw2 = np.asarray(inputs["ffn_w2"], np.float32)
    # fw2[(b*FPB+j)*128+p, ho*128+h] -> [ho, b, p, j, h]
    fw2_t = np.ascontiguousarray(
        fw2.reshape(NBLK, FPB, 128, KO, 128).transpose(3, 0, 2, 1, 4)
    ).astype(np.float16)

    common = {
        "pe": pe_t,
        "adj": np.ascontiguousarray(np.asarray(inputs["causal_adjacency"], np.float32)),
        "w1a": w1a,
        "w1b": w1b,
        "b1f": np.ascontiguousarray(b1f),
        "w2bd": w2bd,
        "b2s": np.ascontiguousarray(b2s),
        "pr": np.ascontiguousarray(pr_f).astype(np.float16),
        "prw1": prw1,
        "fw1": fw1_t,
        "fb1": np.ascontiguousarray(
            np.asarray(inputs["ffn_b1"], np.float32).reshape(FO, 128).T
        ),
        "fw2": fw2_t,
        "fb2": np.ascontiguousarray(
            np.asarray(inputs["ffn_b2"], np.float32).reshape(KO, 128).T
        ),
    }
    toks = hs.reshape(NTOK, H)
    in_maps = []
    for c in range(N_CORES):
        m = dict(common)
        m["xtd"] = np.ascontiguousarray(toks[c * T:(c + 1) * T].T).astype(
            np.float16
        )
        in_maps.append(m)
    return in_maps


def run(inputs, trace=False):
    """Returns (full output [B,S,H] fp32, BassKernelResults)."""
    if "nc" not in _CACHE:
        _CACHE["nc"] = _build()
    nc = _CACHE["nc"]
    in_maps = _prep(inputs)
    res = run_bass_kernel_spmd(
        nc, in_maps, core_ids=list(range(N_CORES)), trace=trace
    )
    full = np.empty((NTOK, H), np.float32)
    for c in range(N_CORES):
        full[c * T:(c + 1) * T] = res.results[c]["out"].T
    return full.reshape(B, S, H), res


def kernel(**inputs):
    full, _ = run(inputs, trace=False)
    return full


# revision 28
# speedup vs baseline: 1.0736x; 1.0008x over previous
"""Trainium2 Bass kernel for nn_CausalMoE.

Reference computation (B=2, S=2048, H=2048, G=16, GH=8, FFN=8192):
  cv        = tanh(hs @ P_extract)                        [N,G]   N = B*S = 4096
  pi        = cv @ A                                      [N,G]
  h[:,m,:]  = cv @ W1[m,:G,:] + pi[:,m,None]*W1[m,G,:] + b1[m]
  h         = gelu(h)  (exact erf gelu)                   [N,G,GH]
  effects   = sum_k h[:,m,k] W2[m,k] + b2[m]              [N,G]
  modified  = hs + 0.5 * effects @ P_route                [N,H]
  ffn_h     = gelu(modified @ ffn_w1 + ffn_b1)            [N,F]
  out       = ffn_h @ ffn_w2 + ffn_b2                     [N,H]

Strategy: pure data-parallel over the 8 NeuronCores (512 tokens/core),
weights replicated.  Everything is computed feature-major (activations
stored transposed, [feature, token]) so every matmul has its contraction
dim on partitions with weights as the stationary operand; the host
shards hs transposed and the gather transposes the output back, so the
kernel needs no on-chip transposes.  The FFN runs entirely in fp16
(e5m10): same 1-cycle/row PE speed class as bf16, FWL weight loads
(97 ns vs 187 ns for fp32 -- the f32r stream was weight-load bound at
227 ns/MM; fp16 hits the 216 ns N=512 issue floor), half the weight DMA
of f32r, and ~11-bit mantissas keep the max rel err ~6e-3.  The tiny
causal-mechanism chain stays f32r (its weights are folded into three
small matmuls via host-side restructuring).

The serial prologue is eliminated with a low-rank split of FFN layer 1:
modified @ ffn_w1 = hs @ ffn_w1 + effects @ (0.5 P_route @ ffn_w1),
where prw1 = P_route @ ffn_w1 [16, 8192] is precomputed on the host.
The first E=4 F-tiles of FFN1 run on raw hs (their rank-16 correction is
one K=16 matmul into the same PSUM accumulation before the gelu), so the
big matmul stream starts the moment the first x chunk + weight tile land
(~1 us) and the extraction/mechanism chain, routing matmuls and
modified-x evictions all hide inside it.  modified is written to a
separate tile (xT stays read-only) so the DVE evictions have no
ordering hazard against the raw-x matmuls.  FFN1 of block b+1 is
interleaved with FFN2 of block b to cover the last-gelu ACT latency at
each block boundary.  No PE idle gaps => the HAM clock gate stays at
2.4 GHz after the initial ~3.4 us ramp, with no junk keepalive matmuls.
"""
import sys

sys.path.insert(0, "/opt/trn_rl_repo")

import numpy as np

import concourse.bacc as bacc
import concourse.mybir as mybir
import concourse.tile as tile
from concourse.bass_utils import run_bass_kernel_spmd

F32 = mybir.dt.float32
F32R = mybir.dt.float32r
F16 = mybir.dt.float16
AF = mybir.ActivationFunctionType

B, S, H = 2, 2048, 2048
G, GH, F = 16, 8, 8192
N_CORES = 8
NTOK = B * S              # 4096 tokens total
T = NTOK // N_CORES       # 512 tokens per core
KO = H // 128             # 16 contraction tiles over H
FO = F // 128             # 64 F tiles
NBLK = 4                  # F blocks
FPB = FO // NBLK          # 16 F tiles per block
NRAW = 6                  # leading F-tiles computed on raw hs + rank-16 fixup

_CACHE = {}


def _build():
    nc = bacc.Bacc("TRN2", target_bir_lowering=False, debug=False)
    # host-side shard layout: xtd = hs_shard.T  [H, T] (feature-major, fp16)
    xtd = nc.dram_tensor("xtd", [H, T], F16, kind="ExternalInput").ap()
    pe = nc.dram_tensor("pe", [128, KO, G], F16, kind="ExternalInput").ap()
    adj = nc.dram_tensor("adj", [G, G], F32, kind="ExternalInput").ap()
    w1a = nc.dram_tensor("w1a", [G, G * GH], F32, kind="ExternalInput").ap()
    w1b = nc.dram_tensor("w1b", [G, G * GH], F32, kind="ExternalInput").ap()
    b1f = nc.dram_tensor("b1f", [G * GH, 1], F32, kind="ExternalInput").ap()
    w2bd = nc.dram_tensor("w2bd", [G * GH, G], F32, kind="ExternalInput").ap()
    b2s = nc.dram_tensor("b2s", [G, 1], F32, kind="ExternalInput").ap()
    pr = nc.dram_tensor("pr", [G, H], F16, kind="ExternalInput").ap()
    # prw1 = P_route @ ffn_w1, [g, fo, f]; correction weights for raw tiles
    prw1 = nc.dram_tensor("prw1", [G, NRAW, 128], F16, kind="ExternalInput").ap()
    # host-retiled: fw1t[fo, p, ko, f] = ffn_w1[ko*128+p, fo*128+f]  (fp16)
    fw1 = nc.dram_tensor("fw1", [FO, 128, KO, 128], F16, kind="ExternalInput").ap()
    fb1 = nc.dram_tensor("fb1", [128, FO], F32, kind="ExternalInput").ap()
    # host-retiled: fw2t[ho, b, p, j, h] = ffn_w2[(b*FPB+j)*128+p, ho*128+h]
    fw2 = nc.dram_tensor(
        "fw2", [KO, NBLK, 128, FPB, 128], F16, kind="ExternalInput"
    ).ap()
    fb2 = nc.dram_tensor("fb2", [128, KO], F32, kind="ExternalInput").ap()
    # output stays feature-major [H, T]; the host gather transposes
    out = nc.dram_tensor("out", [H, T], F32, kind="ExternalOutput").ap()

    with tile.TileContext(nc) as tc:
        with (
            tc.tile_pool(name="const", bufs=1) as const,
            tc.tile_pool(name="xt", bufs=1) as xtp,
            tc.tile_pool(name="mod", bufs=1) as modp,
            tc.tile_pool(name="h1", bufs=2) as h1p,
            tc.tile_pool(name="oacc", bufs=1) as oap,
            tc.tile_pool(name="w1", bufs=6) as w1p,
            tc.tile_pool(name="w2", bufs=5) as w2p,
            tc.tile_pool(name="sm", bufs=1) as smp,
            tc.tile_pool(name="mm", bufs=4, space="PSUM") as mmp,
            tc.tile_pool(name="md", bufs=2, space="PSUM") as mdp,
            tc.tile_pool(name="ch", bufs=2, space="PSUM") as chp,
        ):
            # explicit zero tile for activation biases: a float bias would
            # synthesize a const-AP pool whose TENSOR_LOAD sits in the
            # serialized kernel preamble (~2.7us)
            zz = const.tile([G, 1], F32)
            nc.gpsimd.memset(zz[:], 0.0)

            # warm the ACT Tanh+Gelu LUTs while DMAs land, so the ~1.3us
            # table loads are off the small-chain critical path
            act_warm = const.tile([1, 2], F32)
            nc.scalar.activation(act_warm[:, 0:1], zz[0:1, :], AF.Tanh,
                                 bias=zz[0:1, :])
            nc.scalar.activation(act_warm[:, 1:2], zz[0:1, :], AF.Gelu,
                                 bias=zz[0:1, :])

            # PE clock warm-up: the DMA queues take ~8.5us to deliver their
            # first payloads, during which the PE would otherwise idle cold
            # (HAM gates it to 1.2 GHz until ~3.4us of sustained activity).
            # A couple of junk fp16 matmuls on a zeroed tile bridge the gap
            # between PE-queue start and the first x chunk landing.
            scr = const.tile([128, T], F16)
            nc.gpsimd.memset(scr[:], 0.0)
            jp = mmp.tile([128, T], F32, tag="mm")
            for _ in range(7):
                nc.tensor.matmul(
                    jp[:], scr[:, 0:128], scr[:], start=True, stop=True
                )

            # small consts on the gpsimd queue (pe first -- the first real
            # matmul needs it)
            pe_sb = const.tile([128, KO, G], F16)
            nc.gpsimd.dma_start(pe_sb[:], pe)
            adj_sb = const.tile([G, G], F32R)
            nc.gpsimd.dma_start(adj_sb[:], adj.bitcast(F32R))
            w1a_sb = const.tile([G, G * GH], F32R)
            nc.gpsimd.dma_start(w1a_sb[:], w1a.bitcast(F32R))
            w1b_sb = const.tile([G, G * GH], F32R)
            nc.gpsimd.dma_start(w1b_sb[:], w1b.bitcast(F32R))
            b1f_sb = const.tile([G * GH, 1], F32)
            nc.gpsimd.dma_start(b1f_sb[:], b1f)
            w2bd_sb = const.tile([G * GH, G], F32R)
            nc.gpsimd.dma_start(w2bd_sb[:], w2bd.bitcast(F32R))
            b2s_sb = const.tile([G, 1], F32)
            nc.gpsimd.dma_start(b2s_sb[:], b2s)
            fb1_sb = const.tile([128, FO], F32)
            nc.gpsimd.dma_start(fb1_sb[:], fb1)
            fb2_sb = const.tile([128, KO], F32)
            nc.gpsimd.dma_start(fb2_sb[:], fb2)

            # ---- feature-major xT [128, KO, T] ----
            # Read-only for the whole kernel: raw tiles + routing read it;
            # modified goes to a separate tile, so no version hazards.
            # The early phase is HBM-bandwidth-bound (~3.5 MiB of x + leading
            # weights at ~300 GB/s), so ALL critical payload rides ONE fast
            # queue in exact consumption order; the tiny routing weights go
            # on the otherwise-idle scalar queue and consts on gpsimd.
            xT = xtp.tile([128, KO, T], F16)
            xtd_t = xtd.rearrange("(ko p) t -> p ko t", p=128)
            wts = {}

            def wt_dma(fo):
                wts[fo] = w1p.tile(
                    [128, KO, 128], F16, tag="w1", name=f"wt{fo}"
                )
                nc.sync.dma_start(wts[fo][:], fw1[fo])

            nc.sync.dma_start(xT[:, 0:2, :], xtd_t[:, 0:2, :])
            wt_dma(0)
            nc.sync.dma_start(xT[:, 2:4, :], xtd_t[:, 2:4, :])
            wt_dma(1)
            nc.sync.dma_start(xT[:, 4:6, :], xtd_t[:, 4:6, :])
            nc.sync.dma_start(xT[:, 6:8, :], xtd_t[:, 6:8, :])
            wt_dma(2)
            nc.sync.dma_start(xT[:, 8:10, :], xtd_t[:, 8:10, :])
            nc.sync.dma_start(xT[:, 10:12, :], xtd_t[:, 10:12, :])
            nc.sync.dma_start(xT[:, 12:14, :], xtd_t[:, 12:14, :])
            nc.sync.dma_start(xT[:, 14:16, :], xtd_t[:, 14:16, :])
            wt_dma(3)
            wt_dma(4)
            wt_dma(5)

            # routing / correction weights (needed ~25us+) on the scalar queue
            pr_sb = const.tile([G, H], F16)
            nc.scalar.dma_start(pr_sb[:], pr)
            prw1_sb = const.tile([G, NRAW, 128], F16)
            nc.scalar.dma_start(prw1_sb[:], prw1)

            modT = modp.tile([128, KO, T], F16)
            out_acc = oap.tile([128, KO, T], F32R)
            out_t = out.bitcast(F32R).rearrange("(ho p) t -> p ho t", p=128)

            h1b = {0: h1p.tile([128, FPB, T], F16, tag="h1", name="h1b0")}
            pfs = {}

            def ffn1_mm(fo, ko, last):
                # one K-tile of FFN1 tile fo; raw tiles keep the group open
                # for the later rank-16 correction
                src = xT if fo < NRAW else modT
                if ko == 0:
                    pfs[fo] = mmp.tile([128, T], F32, tag="mm", name=f"pf{fo}")
                nc.tensor.matmul(
                    pfs[fo][:], wts[fo][:, ko, :], src[:, ko, :],
                    start=(ko == 0), stop=(last and fo >= NRAW),
                )

            def ffn1_fixup(fo):
                # rank-16 correction: + prw1[fo]^T @ (0.5*effects^T), closing
                # the accumulation group, then evict through the exact gelu
                nc.tensor.matmul(
                    pfs[fo][:], prw1_sb[:, fo, :], effs_sb[:],
                    start=False, stop=True,
                )
                ffn1_gelu(fo)

            def ffn1_gelu(fo):
                nc.scalar.activation(
                    h1b[fo // FPB][:, fo % FPB, :], pfs[fo][:], AF.Gelu,
                    bias=fb1_sb[:, fo:fo + 1],
                )
                del pfs[fo]

            # ---- P1: wavefront over arriving x chunks ----
            # Chunk c (kos 2c, 2c+1) unlocks: extraction of those kos, plus
            # fo0/fo1/fo2 matmuls lagged 1/2/3 chunks behind, so the PE
            # always has landed data queued and consumption (~8 MMs/chunk)
            # matches the DMA arrival rate -- no starvation stall, and the
            # HAM clock gate stays engaged once warm.
            cv_ps = chp.tile([128, T], F32, tag="ch")
            for c in range(8):
                # lagged FFN1 work first: it uses chunks already resident, so
                # the PE only blocks on chunk c at the batch's end
                for lag, fo in ((1, 0), (2, 1), (3, 2)):
                    cb = c - lag
                    if cb >= 0:
                        ffn1_mm(fo, 2 * cb, last=False)
                        ffn1_mm(fo, 2 * cb + 1, last=False)
                for ko in (2 * c, 2 * c + 1):
                    nc.tensor.matmul(
                        cv_ps[0:G, :], pe_sb[:, ko, :], xT[:, ko, :],
                        start=(ko == 0), stop=(ko == KO - 1),
                    )
            cvt_sb = smp.tile([G, T], F32R, tag="cv")
            nc.scalar.activation(cvt_sb[:], cv_ps[0:G, :], AF.Tanh,
                                 bias=zz[:])

            # ---- P2: wavefront tails with the mechanism chain sprinkled in
            # (each chain matmul sits behind enough FFN1 work to hide the
            # ACT/DVE latency of the step feeding it)
            ffn1_mm(0, 14, last=False)
            ffn1_mm(0, 15, last=False)
            pi_ps = chp.tile([128, T], F32, tag="ch")
            nc.tensor.matmul(
                pi_ps[0:G, :], adj_sb[:], cvt_sb[:], start=True, stop=True
            )
            pit_sb = smp.tile([G, T], F32R, tag="pi")
            nc.vector.tensor_copy(pit_sb[:], pi_ps[0:G, :])
            ffn1_mm(1, 12, last=False)
            ffn1_mm(1, 13, last=False)
            h_ps = chp.tile([128, T], F32, tag="ch")
            nc.tensor.matmul(h_ps[:], w1a_sb[:], cvt_sb[:], start=True, stop=False)
            nc.tensor.matmul(h_ps[:], w1b_sb[:], pit_sb[:], start=False, stop=True)
            ffn1_mm(1, 14, last=False)
            ffn1_mm(1, 15, last=False)
            hm_sb = smp.tile([G * GH, T], F32R, tag="hm")
            nc.scalar.activation(hm_sb[:], h_ps[:], AF.Gelu, bias=b1f_sb[:])
            ffn1_mm(2, 10, last=False)
            ffn1_mm(2, 11, last=False)
            eff_ps = chp.tile([128, T], F32, tag="ch")
            nc.tensor.matmul(
                eff_ps[0:G, :], w2bd_sb[:], hm_sb[:], start=True, stop=True
            )
            for ko in range(12, 16):
                ffn1_mm(2, ko, last=False)
            # bias-add on DVE: keeps the ACT LUT on Gelu (no table reload)
            effs_sb = smp.tile([G, T], F16, tag="eff")
            nc.vector.tensor_scalar_add(effs_sb[:], eff_ps[0:G, :], b2s_sb[:])

            # ---- P4: corrections for fo0..2 ----
            for fo in range(3):
                ffn1_fixup(fo)

            # ---- P5: routing matmuls 1:2 with fo3/fo4 raw (the modified-x
            # DVE evictions run at ~550ns each vs 432ns for a 1:1 interleave,
            # so two raw streams keep the PE ahead of the eviction rate) ----
            for ho in range(KO):
                md = mdp.tile([128, T], F32, tag="md")
                nc.tensor.matmul(
                    md[:], pr_sb[:, ho * 128:(ho + 1) * 128], effs_sb[:],
                    start=True, stop=True,
                )
                # three raw streams (864ns of matmuls) per iteration: one
                # DVE eviction is ~850ns incl. semaphore latency, so fewer
                # would leave the PE paced by the evictions (gpsimd cannot
                # read PSUM, so the adds cannot be split across engines)
                ffn1_mm(3, ho, last=False)
                ffn1_mm(4, ho, last=False)
                ffn1_mm(5, ho, last=False)
                nc.vector.tensor_add(modT[:, ho, :], xT[:, ho, :], md[:])
            ffn1_fixup(3)
            ffn1_fixup(4)
            ffn1_fixup(5)

            # ---- P6: rest of block 0 on modified x ----
            for fo in range(NRAW, FPB):
                wt_dma(fo)
                for ko in range(KO):
                    ffn1_mm(fo, ko, last=(ko == KO - 1))
                ffn1_gelu(fo)

            # ---- P7: FFN2 of block b interleaved with FFN1 of block b+1 ----
            for b in range(NBLK):
                if b + 1 < NBLK:
                    h1b[b + 1] = h1p.tile([128, FPB, T], F16, tag="h1", name=f"h1b{b+1}")
                for k in range(FPB):
                    if b + 1 < NBLK:
                        fo = (b + 1) * FPB + k
                        wts[fo] = w1p.tile([128, KO, 128], F16, tag="w1", name=f"wt{fo}")
                        nc.sync.dma_start(wts[fo][:], fw1[fo])
                        for ko in range(KO):
                            ffn1_mm(fo, ko, last=(ko == KO - 1))
                        ffn1_gelu(fo)
                    ho = k
                    w2t = w2p.tile([128, FPB, 128], F16, tag="w2")
                    nc.sync.dma_start(w2t[:], fw2[ho, b])
                    po = mmp.tile([128, T], F32, tag="mm")
                    for j in range(FPB):
                        nc.tensor.matmul(
                            po[:], w2t[:, j, :], h1b[b][:, j, :],
                            start=(j == 0), stop=(j == FPB - 1),
                        )
                    if b == 0:
                        nc.vector.tensor_scalar_add(
                            out_acc[:, ho, :], po[:], fb2_sb[:, ho:ho + 1]
                        )
                    else:
                        nc.vector.tensor_add(
                            out_acc[:, ho, :], out_acc[:, ho, :], po[:]
                        )
                    if b == NBLK - 1:
                        # store this H-tile feature-major; host transposes
                        nc.sync.dma_start(out_t[:, ho, :], out_acc[:, ho, :])

    nc.compile()
    return nc


def _prep(inputs):
    """Host-side restructuring of weights + sharding."""
    import ml_dtypes  # noqa: F401  (np.float16 used directly)

    hs = np.ascontiguousarray(np.asarray(inputs["hidden_states"], np.float32))
    W1 = np.asarray(inputs["W1"], np.float32)
    b1 = np.asarray(inputs["b1"], np.float32)
    W2 = np.asarray(inputs["W2"], np.float32)
    b2 = np.asarray(inputs["b2"], np.float32)

    w1a = np.ascontiguousarray(
        W1[:, :G, :].transpose(1, 0, 2).reshape(G, G * GH)
    )
    w1b = np.zeros((G, G * GH), np.float32)
    for m in range(G):
        w1b[m, m * GH:(m + 1) * GH] = W1[m, G, :]
    b1f = b1.reshape(G * GH, 1)
    w2bd = np.zeros((G * GH, G), np.float32)
    for m in range(G):
        w2bd[m * GH:(m + 1) * GH, m] = 0.5 * W2[m, :]
    b2s = (0.5 * b2).reshape(G, 1)

    pe = np.asarray(inputs["P_extract"], np.float32)
    # pe[h, g] -> [p, ko, g] with h = ko*128 + p
    pe_t = np.ascontiguousarray(
        pe.reshape(KO, 128, G).transpose(1, 0, 2)
    ).astype(np.float16)

    pr_f = np.asarray(inputs["P_route"], np.float32)
    fw1 = np.asarray(inputs["ffn_w1"], np.float32)
    # rank-16 correction weights for the raw leading tiles:
    # prw1[g, fo, f] = (P_route @ ffn_w1)[g, fo*128+f]
    prw1 = np.ascontiguousarray(
        (pr_f @ fw1[:, : NRAW * 128]).reshape(G, NRAW, 128)
    ).astype(np.float16)
    # fw1[ko*128+p, fo*128+f] -> [fo, p, ko, f]
    fw1_t = np.ascontiguousarray(
        fw1.reshape(KO, 128, FO, 128).transpose(2, 1, 0, 3)
    ).astype(np.float16)
    fw2 = np.asarray(inputs["ffn_w2"], np.float32)
    # fw2[(b*FPB+j)*128+p, ho*128+h] -> [ho, b, p, j, h]
    fw2_t = np.ascontiguousarray(
        fw2.reshape(NBLK, FPB, 128, KO, 128).transpose(3, 0, 2, 1, 4)
    ).astype(np.float16)

    common = {
        "pe": pe_t,
        "adj": np.ascontiguousarray(np.asarray(inputs["causal_adjacency"], np.float32)),
        "w1a": w1a,
        "w1b": w1b,
        "b1f": np.ascontiguousarray(b1f),
        "w2bd": w2bd,
        "b2s": np.ascontiguousarray(b2s),
        "pr": np.ascontiguousarray(pr_f).astype(np.float16),
        "prw1": prw1,
        "fw1": fw1_t,
        "fb1": np.ascontiguousarray(
            np.asarray(inputs["ffn_b1"], np.float32).reshape(FO, 128).T
        ),
        "fw2": fw2_t,
        "fb2": np.ascontiguousarray(
            np.asarray(inputs["ffn_b2"], np.float32).reshape(KO, 128).T
        ),
    }
    toks = hs.reshape(NTOK, H)
    in_maps = []
    for c in range(N_CORES):
        m = dict(common)
        m["xtd"] = np.ascontiguousarray(toks[c * T:(c + 1) * T].T).astype(
            np.float16
        )
        in_maps.append(m)
    return in_maps


def run(inputs, trace=False):
    """Returns (full output [B,S,H] fp32, BassKernelResults)."""
    if "nc" not in _CACHE:
        _CACHE["nc"] = _build()
    nc = _CACHE["nc"]
    in_maps = _prep(inputs)
    res = run_bass_kernel_spmd(
        nc, in_maps, core_ids=list(range(N_CORES)), trace=trace
    )
    full = np.empty((NTOK, H), np.float32)
    for c in range(N_CORES):
        full[c * T:(c + 1) * T] = res.results[c]["out"].T
    return full.reshape(B, S, H), res


def kernel(**inputs):
    full, _ = run(inputs, trace=False)
    return full
